# revision 8
# baseline (speedup 1.0000x reference)
"""MetaBaseline (retrieval_knn) Trainium2 kernel.

Problem: E=256 episodes; per episode:
  shot_sum[W,D], shot_mean = mean over S shots
  dist[W,Q]   = ||shot_mean_w - q_q||_2
  weights     = softmax(-dist, axis=Q)
  pooled[W,D] = weights @ x_query
  proto       = l2norm(shot_sum + 37*pooled)       (the /42 cancels in l2norm)
  logits[Q,W] = temp * l2norm(x_query) @ proto.T

Sharding: pure data parallel over E across 8 NeuronCores (32 episodes/core).
On-device layout: blocks of 4 episodes packed on the partition dim at
32-partition offsets (col-tiled matmuls), so softmax/activation work runs on
[128, Q] tiles serving 4 episodes at once.

v2 design (vs v1): x_query ships ONCE (transposed layout qT only, 9.8MB/core);
the natural-layout copy needed by the pooled matmul is produced on-chip with
PE transposes (48 [128,<=128] transposes/block) + ACT/DVE psum->sbuf copies.
nq2 (per-query norms) broadcast into the dist psum via a K=8 selector matmul
(bf16 hi+lo rows for f32-grade accuracy); shot_sum added into the proto psum
via a K=80 selector matmul of packed rows. Software pipeline uses a 4-deep
skew so the PE never waits on the serial ACT softmax/l2norm chains:

  iter t PE order: G_t | qnatT_t (48 tp) | M3_{t-3} | wT_{t-1} | pT_{t-2} | M2_{t-1}
  ACT during t:    softmax_t (Ln/Exp/Exp+accum), l2norm_{t-1}, lgsb_{t-3}, half qn copies
  DVE during t:    recip_t, w37_t, proton_{t-1}, wT/pT copies, half qn copies
  GPSIMD:          outT DMA (t-3)

Per-core DMA ~12.9MB (36us floor at 358GB/s); PE ~5us/block warm.
Host does cheap O(N*D) prep (sums/norms/layout packs) and the final
transpose + temp/||q|| scaling of the raw logits.
"""
import sys

sys.path.insert(0, "/opt/trn_rl_repo")

import numpy as np
import ml_dtypes

import concourse.bass as bass
import concourse.tile as tile
from concourse import bacc, mybir
from concourse.bass_utils import run_bass_kernel_spmd
from concourse.masks import make_identity

bf16 = mybir.dt.bfloat16
f32 = mybir.dt.float32

E, W, S, Q, D = 256, 20, 5, 300, 512
ALPHA = 37.0
NCORES = 8
EL = E // NCORES      # 32 episodes per core
BLK = 4               # episodes per block (packed at 32-partition offsets)
NBLK = EL // BLK      # 8 blocks
DC = D // 128         # 4 K-chunks over D
QCH = [(0, 128), (128, 128), (256, Q - 256)]  # q chunks (offset, count)

_BUILT = None


def _pin_act_table_set():
    """Make Bacc's ACT-table-load pass pick one covering set for Ln/Exp/Square.

    The pass walks activations and loads the first set containing the needed
    function; Ln's first set lacks Exp and vice versa, so alternating
    Ln/Exp/Square thrashes ACT_TABLE_LOAD (~1.3us each). Hide those functions
    from every set except natural_log_exp_and_others (set *indices* are
    preserved — contents of the real act_info.json are untouched).
    """
    import concourse.bacc as bacc_mod
    from concourse import hw_specs

    if getattr(bacc_mod, "_act_tables_pinned", False):
        return
    orig = hw_specs.get_activation_tables
    pin = {
        mybir.ActivationFunctionType.Ln,
        mybir.ActivationFunctionType.Exp,
        mybir.ActivationFunctionType.Square,
    }
    keep = "natural_log_exp_and_others"

    def pinned(arch):
        tabs = orig(arch)
        return {
            name: set(fns) if name == keep else (set(fns) - pin)
            for name, fns in tabs.items()
        }

    bacc_mod.get_activation_tables = pinned
    bacc_mod._act_tables_pinned = True


def _build():
    _pin_act_table_set()
    nc = bacc.Bacc("TRN2", target_bir_lowering=False, debug=False)

    qT = nc.declare_dram_parameter("qT", [128, NBLK, DC * BLK * Q], bf16, isOutput=False)
    mTs = nc.declare_dram_parameter("mTs", [128, NBLK, DC * BLK * W], bf16, isOutput=False)
    ssum_p = nc.declare_dram_parameter("ssum_p", [BLK * W, NBLK, D], bf16, isOutput=False)
    nq2hl = nc.declare_dram_parameter("nq2hl", [2 * BLK, NBLK, Q], bf16, isOutput=False)
    nm2b = nc.declare_dram_parameter("nm2b", [128, NBLK], f32, isOutput=False)
    sel8 = nc.declare_dram_parameter("sel8", [2 * BLK, 128], bf16, isOutput=False)
    sel80 = nc.declare_dram_parameter("sel80", [BLK * W, 128], bf16, isOutput=False)
    outT = nc.declare_dram_parameter("outT", [128, NBLK, Q], bf16, isOutput=True)

    with tile.TileContext(nc) as tc:
        with tc.tile_pool(name="const", bufs=1) as const, \
             tc.tile_pool(name="inp", bufs=2) as inp, \
             tc.tile_pool(name="qn", bufs=2) as qnp, \
             tc.tile_pool(name="mid", bufs=2) as mid, \
             tc.tile_pool(name="psG", bufs=2, space="PSUM") as psG, \
             tc.tile_pool(name="psPR", bufs=2, space="PSUM") as psPR, \
             tc.tile_pool(name="psLG", bufs=1, space="PSUM") as psLG, \
             tc.tile_pool(name="psQT", bufs=2, space="PSUM") as psQT, \
             tc.tile_pool(name="psT", bufs=1, space="PSUM") as psT:

            # ---- constants (loaded once) ----
            nm2b_t = const.tile([128, NBLK], f32)
            nc.sync.dma_start(out=nm2b_t, in_=nm2b[:, :])
            sel8_t = const.tile([2 * BLK, 128], bf16)
            nc.sync.dma_start(out=sel8_t, in_=sel8[:, :])
            sel80_t = const.tile([BLK * W, 128], bf16)
            nc.sync.dma_start(out=sel80_t, in_=sel80[:, :])
            ident = const.tile([128, 128], bf16)
            make_identity(nc, ident)
            # PE warmup: dense dummy matmuls during the DMA ramp so the HAM
            # un-throttles (1.2 -> 2.4 GHz) before the first real block.
            wups = psLG.tile([128, 128], f32, tag="lgT")
            for _ in range(30):
                nc.tensor.matmul(wups, ident, ident, start=True, stop=True)
            wupd = const.tile([128, 128], bf16)
            nc.vector.tensor_copy(wupd, wups)

            def s_dma(b, st):
                """input DMAs for block b."""
                qTt = inp.tile([128, DC, BLK, Q], bf16, tag="qTt", bufs=4)
                nc.sync.dma_start(
                    out=qTt,
                    in_=qT[:, b, :].rearrange("p (c j q) -> p c j q", c=DC, j=BLK),
                )
                mTs_t = inp.tile([128, DC, BLK, W], bf16, tag="mTs", bufs=3)
                nc.sync.dma_start(
                    out=mTs_t,
                    in_=mTs[:, b, :].rearrange("p (c j w) -> p c j w", c=DC, j=BLK),
                )
                ssum_t = inp.tile([BLK * W, D], bf16, tag="ssum", bufs=3)
                nc.sync.dma_start(out=ssum_t, in_=ssum_p[:, b, :])
                nq2_t = inp.tile([2 * BLK, Q], bf16, tag="nq2", bufs=3)
                nc.sync.dma_start(out=nq2_t, in_=nq2hl[:, b, :])
                st.update(qTt=qTt, mTs_t=mTs_t, ssum_t=ssum_t, nq2_t=nq2_t)

            def s_g(b, st):
                """G psum = nq2 (K=8 selector) - 2*mean.T @ q, col-tiled."""
                qTt, mTs_t, nq2_t = st["qTt"], st["mTs_t"], st["nq2_t"]
                g = psG.tile([128, Q], f32, tag="g")
                for j in range(BLK):
                    nc.tensor.matmul(
                        g[32 * j:32 * j + 32, :],
                        sel8_t[:, 32 * j:32 * j + 32], nq2_t,
                        start=True, stop=False,
                        tile_position=(0, 32 * j),
                    )
                    for c in range(DC):
                        nc.tensor.matmul(
                            g[32 * j:32 * j + W, :],
                            mTs_t[:, c, j, :], qTt[:, c, j, :],
                            start=False, stop=(c == DC - 1),
                            tile_position=(0, 32 * j),
                        )
                st["g"] = g

            def s_softmax(b, st):
                """dist chain from G psum; w37 bf16 [128, Q]."""
                g = st["g"]
                lnv = mid.tile([128, Q], f32, tag="lnv")
                nc.scalar.activation(out=lnv, in_=g,
                                     func=mybir.ActivationFunctionType.Ln,
                                     bias=nm2b_t[:, b:b + 1], scale=1.0)
                dist = mid.tile([128, Q], f32, tag="dist")
                nc.scalar.activation(out=dist, in_=lnv,
                                     func=mybir.ActivationFunctionType.Exp,
                                     bias=0.0, scale=0.5)
                wexp = mid.tile([128, Q], f32, tag="wexp")
                sums = mid.tile([128, 1], f32, tag="sums")
                nc.scalar.activation(out=wexp, in_=dist,
                                     func=mybir.ActivationFunctionType.Exp,
                                     bias=0.0, scale=-1.0, accum_out=sums)
                recip = mid.tile([128, 1], f32, tag="recip")
                nc.vector.reciprocal(recip, sums)
                w37 = mid.tile([128, Q], bf16, tag="w37")
                nc.vector.tensor_scalar(
                    out=w37, in0=wexp, scalar1=recip, scalar2=ALPHA,
                    op0=mybir.AluOpType.mult, op1=mybir.AluOpType.mult,
                )
                st["w37"] = w37

            def s_qtrans(b, st):
                """on-chip transpose qT -> qnat chunks for the pooled matmul."""
                qTt = st["qTt"]
                qn_t = []
                for ci, (q0, cnt) in enumerate(QCH):
                    t = qnp.tile([128, BLK, D], bf16, tag=f"qn{ci}")
                    qn_t.append(t)
                st["qn_t"] = qn_t
                k = 0
                for j in range(BLK):
                    for ci, (q0, cnt) in enumerate(QCH):
                        tp = psQT.tile([128, DC, 128], bf16, tag="qtp")
                        for c in range(DC):
                            nc.tensor.transpose(
                                tp[0:cnt, c, :], qTt[:, c, j, q0:q0 + cnt], ident)
                        tpf = tp[0:cnt, :, :].rearrange("p c d -> p (c d)")
                        # alternate copy engine to split the psum->sbuf load
                        if k % 2 == 0:
                            nc.scalar.copy(qn_t[ci][0:cnt, j, :], tpf)
                        else:
                            nc.vector.tensor_copy(qn_t[ci][0:cnt, j, :], tpf)
                        k += 1

            def s_wt(b, st):
                """wT transposes + copies."""
                w37 = st["w37"]
                wTps = psT.tile([128, len(QCH), 128], bf16, tag="tp")
                for ci, (q0, cnt) in enumerate(QCH):
                    nc.tensor.transpose(wTps[0:cnt, ci, :], w37[:, q0:q0 + cnt], ident)
                wTsb = []
                for ci, (q0, cnt) in enumerate(QCH):
                    t = mid.tile([128, 128], bf16, tag=f"wTsb{ci}")
                    nc.vector.tensor_copy(t[0:cnt], wTps[0:cnt, ci, :])
                    wTsb.append(t)
                st["wTsb"] = wTsb

            def s_m2(b, st):
                """proto accumulation: ssum (K=80 selector) + 37*pooled; l2norm."""
                wTsb, qn_t, ssum_t = st["wTsb"], st["qn_t"], st["ssum_t"]
                pr = psPR.tile([128, D], f32, tag="pr")
                for j in range(BLK):
                    nc.tensor.matmul(
                        pr[32 * j:32 * j + 32, :],
                        sel80_t[:, 32 * j:32 * j + 32], ssum_t,
                        start=True, stop=False,
                        tile_position=(0, 32 * j),
                    )
                    for ci, (q0, cnt) in enumerate(QCH):
                        nc.tensor.matmul(
                            pr[32 * j:32 * j + W, :],
                            wTsb[ci][0:cnt, 32 * j:32 * j + W],
                            qn_t[ci][0:cnt, j, :],
                            start=False, stop=(ci == len(QCH) - 1),
                            tile_position=(0, 32 * j),
                        )
                sqdump = mid.tile([128, D], bf16, tag="sqdump")
                n2 = mid.tile([128, 1], f32, tag="n2")
                nc.scalar.activation(out=sqdump, in_=pr,
                                     func=mybir.ActivationFunctionType.Square,
                                     bias=0.0, scale=1.0, accum_out=n2)
                lnn = mid.tile([128, 1], f32, tag="lnn")
                nc.scalar.activation(out=lnn, in_=n2,
                                     func=mybir.ActivationFunctionType.Ln,
                                     bias=0.0, scale=1.0)
                rstd = mid.tile([128, 1], f32, tag="rstd")
                nc.scalar.activation(out=rstd, in_=lnn,
                                     func=mybir.ActivationFunctionType.Exp,
                                     bias=0.0, scale=-0.5)
                proton = mid.tile([128, D], bf16, tag="proton")
                nc.vector.tensor_scalar_mul(out=proton, in0=pr, scalar1=rstd)
                st["proton"] = proton

            def s_pt(b, st):
                """protonT transposes + copies."""
                ptps = psT.tile([128, DC, 128], bf16, tag="tp")
                for c in range(DC):
                    nc.tensor.transpose(ptps[:, c, :], st["proton"][:, 128 * c:128 * (c + 1)], ident)
                ptsb = []
                for c in range(DC):
                    t = mid.tile([128, 128], bf16, tag=f"ptsb{c}")
                    nc.vector.tensor_copy(t, ptps[:, c, :])
                    ptsb.append(t)
                st["ptsb"] = ptsb

            def s_m3(b, st):
                """logits matmul (transposed layout)."""
                ptsb, qTt = st["ptsb"], st["qTt"]
                lgT = psLG.tile([128, Q], f32, tag="lgT")
                for j in range(BLK):
                    for c in range(DC):
                        nc.tensor.matmul(
                            lgT[32 * j:32 * j + W, :],
                            ptsb[c][:, 32 * j:32 * j + W],
                            qTt[:, c, j, :],
                            start=(c == 0), stop=(c == DC - 1),
                            tile_position=(0, 32 * j),
                        )
                st["lgT"] = lgT

            def s_out(b, st):
                """store raw transposed logits; host does final transpose+scale."""
                lgsb = mid.tile([128, Q], bf16, tag="lgsb")
                nc.scalar.copy(lgsb, st["lgT"])
                nc.gpsimd.dma_start(out=outT[:, b, :], in_=lgsb)

            # 4-deep software pipeline; PE never waits on ACT/DVE chains.
            # PE order/iter t: G_t | wT_{t-1} | qnatT_t | M3_{t-3} | pT_{t-2} | M2_{t-1}
            sts = {}
            for t in range(NBLK + 3):
                if t < NBLK:
                    sts[t] = {}
                    s_dma(t, sts[t])
                    s_g(t, sts[t])
                if 0 <= t - 1 < NBLK:
                    s_wt(t - 1, sts[t - 1])
                if t < NBLK:
                    s_softmax(t, sts[t])
                    s_qtrans(t, sts[t])
                if 0 <= t - 3 < NBLK:
                    s_m3(t - 3, sts[t - 3])
                    s_out(t - 3, sts[t - 3])
                if 0 <= t - 2 < NBLK:
                    s_pt(t - 2, sts[t - 2])
                if 0 <= t - 1 < NBLK:
                    s_m2(t - 1, sts[t - 1])
                if 0 <= t - 4 < NBLK:
                    del sts[t - 4]

    nc.finalize()
    return nc


def _get_built():
    global _BUILT
    if _BUILT is None:
        _BUILT = _build()
    return _BUILT


def _prep_core_inputs(x_shot, x_query, temp):
    """x_shot [EL,W,S,D] f32, x_query [EL,Q,D] f32 -> input map for one core."""
    qTr = x_query.transpose(2, 0, 1).reshape(DC, 128, NBLK, BLK, Q)
    qTr = np.ascontiguousarray(qTr.transpose(1, 2, 0, 3, 4)).reshape(128, NBLK, DC * BLK * Q)
    qTr = qTr.astype(ml_dtypes.bfloat16)

    shot_sum = x_shot.sum(axis=2)                    # [EL, W, D] f32
    mean = shot_sum / S
    # ssum packed rows: k = 20*j + w  ->  episode BLK*b+j
    ssp = shot_sum.reshape(NBLK, BLK * W, D).transpose(1, 0, 2)
    ssp = np.ascontiguousarray(ssp).astype(ml_dtypes.bfloat16)
    # mTs[p, b, (c j w)] = -2 * mean[4b+j, w, 128c+p]
    m = (-2.0 * mean).reshape(NBLK, BLK, W, DC, 128)
    m = m.transpose(4, 0, 3, 1, 2).reshape(128, NBLK, DC * BLK * W)
    mTs = np.ascontiguousarray(m).astype(ml_dtypes.bfloat16)

    nq2 = np.einsum("eqd,eqd->eq", x_query.astype(np.float64),
                    x_query.astype(np.float64)).astype(np.float32)   # [EL, Q]
    nq2b = nq2.reshape(NBLK, BLK, Q)                 # [b, j, q]
    hi = nq2b.astype(ml_dtypes.bfloat16)
    lo = (nq2b - hi.astype(np.float32)).astype(ml_dtypes.bfloat16)
    nq2hl = np.concatenate([hi, lo], axis=1)         # [b, 2*BLK, q]
    nq2hl = np.ascontiguousarray(nq2hl.transpose(1, 0, 2))  # [2*BLK, b, q]

    nm2 = np.einsum("ewd,ewd->ew", mean, mean)       # [EL, W] f32
    nm2b = np.zeros((128, NBLK), np.float32)
    for b in range(NBLK):
        for j in range(BLK):
            nm2b[32 * j:32 * j + W, b] = nm2[BLK * b + j]

    return {
        "qT": qTr, "mTs": mTs, "ssum_p": ssp, "nq2hl": nq2hl, "nm2b": nm2b,
    }


def _consts():
    sel8 = np.zeros((2 * BLK, 128), np.float32)
    for j in range(BLK):
        sel8[j, 32 * j:32 * j + W] = 1.0       # hi rows
        sel8[BLK + j, 32 * j:32 * j + W] = 1.0  # lo rows
    sel80 = np.zeros((BLK * W, 128), np.float32)
    for j in range(BLK):
        for w in range(W):
            sel80[W * j + w, 32 * j + w] = 1.0
    return {
        "sel8": sel8.astype(ml_dtypes.bfloat16),
        "sel80": sel80.astype(ml_dtypes.bfloat16),
    }


def _run(x_shot, x_query, temp, trace=False):
    nc = _get_built()
    consts = _consts()
    in_maps = []
    for i in range(NCORES):
        sl = slice(i * EL, (i + 1) * EL)
        m = _prep_core_inputs(x_shot[sl], x_query[sl], temp)
        m.update(consts)
        in_maps.append(m)
    res = run_bass_kernel_spmd(
        nc, in_maps, list(range(NCORES)), trace=trace,
        tmpdir="/tmp/bass_trace_out" if trace else None,
    )
    out = np.empty((E, Q, W), np.float32)
    for i in range(NCORES):
        sl = slice(i * EL, (i + 1) * EL)
        nq2 = np.einsum("eqd,eqd->eq", x_query[sl].astype(np.float64),
                        x_query[sl].astype(np.float64)).astype(np.float32)
        qscale = (np.float32(temp) / np.sqrt(nq2))[:, :, None]   # [EL, Q, 1]
        raw = res.results[i]["outT"].astype(np.float32).reshape(4, 32, NBLK, Q)
        lg = raw[:, 0:W].transpose(2, 0, 1, 3).reshape(EL, W, Q) # [e, w, q]
        out[sl] = lg.transpose(0, 2, 1) * qscale
    return out, res


def kernel(x_shot, x_query, temp):
    x_shot = np.asarray(x_shot, dtype=np.float32)
    x_query = np.asarray(x_query, dtype=np.float32)
    out, _ = _run(x_shot, x_query, np.float32(temp))
    return out


def kernel_timed(x_shot, x_query, temp):
    x_shot = np.asarray(x_shot, dtype=np.float32)
    x_query = np.asarray(x_query, dtype=np.float32)
    out, res = _run(x_shot, x_query, np.float32(temp), trace=True)
    return out, res


# revision 12
# speedup vs baseline: 1.3293x; 1.3293x over previous
"""MetaBaseline (retrieval_knn) Trainium2 kernel.

Problem: E=256 episodes; per episode:
  shot_sum[W,D], shot_mean = mean over S shots
  dist[W,Q]   = ||shot_mean_w - q_q||_2
  weights     = softmax(-dist, axis=Q)
  pooled[W,D] = weights @ x_query
  proto       = l2norm(shot_sum + 37*pooled)       (the /42 cancels in l2norm)
  logits[Q,W] = temp * l2norm(x_query) @ proto.T

Sharding: pure data parallel over E across 8 NeuronCores (32 episodes/core).
On-device layout: blocks of 4 episodes packed on the partition dim at
32-partition offsets (col-tiled matmuls), so softmax/activation work runs on
[128, Q] tiles serving 4 episodes at once.

v2 design (vs v1): x_query ships ONCE (transposed layout qT only, 9.8MB/core);
the natural-layout copy needed by the pooled matmul is produced on-chip with
PE transposes (48 [128,<=128] transposes/block) + ACT/DVE psum->sbuf copies.
nq2 (per-query norms) broadcast into the dist psum via a K=8 selector matmul
(bf16 hi+lo rows for f32-grade accuracy); shot_sum added into the proto psum
via a K=80 selector matmul of packed rows. Software pipeline uses a 4-deep
skew so the PE never waits on the serial ACT softmax/l2norm chains:

  iter t PE order: G_t | qnatT_t (48 tp) | M3_{t-3} | wT_{t-1} | pT_{t-2} | M2_{t-1}
  ACT during t:    softmax_t (Ln/Exp/Exp+accum), l2norm_{t-1}, lgsb_{t-3}, half qn copies
  DVE during t:    recip_t, w37_t, proton_{t-1}, wT/pT copies, half qn copies
  GPSIMD:          outT DMA (t-3)

Per-core DMA ~12.9MB (36us floor at 358GB/s); PE ~5us/block warm.
Host does cheap O(N*D) prep (sums/norms/layout packs) and the final
transpose + temp/||q|| scaling of the raw logits.
"""
import sys

sys.path.insert(0, "/opt/trn_rl_repo")

import numpy as np
import ml_dtypes

import concourse.bass as bass
import concourse.tile as tile
from concourse import bacc, mybir
from concourse.bass_utils import run_bass_kernel_spmd
from concourse.masks import make_identity

bf16 = mybir.dt.bfloat16
f32 = mybir.dt.float32

E, W, S, Q, D = 256, 20, 5, 300, 512
ALPHA = 37.0
NCORES = 8
EL = E // NCORES      # 32 episodes per core
BLK = 4               # episodes per block (packed at 32-partition offsets)
NBLK = EL // BLK      # 8 blocks
DC = D // 128         # 4 K-chunks over D
QCH = [(0, 128), (128, 128), (256, Q - 256)]  # q chunks (offset, count)

_BUILT = None


def _pin_act_table_set():
    """Make Bacc's ACT-table-load pass pick one covering set for Ln/Exp/Square.

    The pass walks activations and loads the first set containing the needed
    function; Ln's first set lacks Exp and vice versa, so alternating
    Ln/Exp/Square thrashes ACT_TABLE_LOAD (~1.3us each). Hide those functions
    from every set except natural_log_exp_and_others (set *indices* are
    preserved — contents of the real act_info.json are untouched).
    """
    import concourse.bacc as bacc_mod
    from concourse import hw_specs

    if getattr(bacc_mod, "_act_tables_pinned", False):
        return
    orig = hw_specs.get_activation_tables
    pin = {
        mybir.ActivationFunctionType.Ln,
        mybir.ActivationFunctionType.Exp,
        mybir.ActivationFunctionType.Square,
    }
    keep = "natural_log_exp_and_others"

    def pinned(arch):
        tabs = orig(arch)
        return {
            name: set(fns) if name == keep else (set(fns) - pin)
            for name, fns in tabs.items()
        }

    bacc_mod.get_activation_tables = pinned
    bacc_mod._act_tables_pinned = True


def _build():
    _pin_act_table_set()
    nc = bacc.Bacc("TRN2", target_bir_lowering=False, debug=False)

    qT = nc.declare_dram_parameter("qT", [128, NBLK, DC * BLK * Q], bf16, isOutput=False)
    mTs = nc.declare_dram_parameter("mTs", [128, NBLK, DC * BLK * W], bf16, isOutput=False)
    ssum_p = nc.declare_dram_parameter("ssum_p", [BLK * W, NBLK, D], bf16, isOutput=False)
    nq2hl = nc.declare_dram_parameter("nq2hl", [2 * BLK, NBLK, Q], bf16, isOutput=False)
    nm2b = nc.declare_dram_parameter("nm2b", [128, NBLK], f32, isOutput=False)
    sel8 = nc.declare_dram_parameter("sel8", [2 * BLK, 128], bf16, isOutput=False)
    sel80 = nc.declare_dram_parameter("sel80", [BLK * W, 128], bf16, isOutput=False)
    outT = nc.declare_dram_parameter("outT", [128, NBLK, Q], bf16, isOutput=True)

    with tile.TileContext(nc) as tc:
        with tc.tile_pool(name="const", bufs=1) as const, \
             tc.tile_pool(name="inp", bufs=2) as inp, \
             tc.tile_pool(name="qn", bufs=2) as qnp, \
             tc.tile_pool(name="mid", bufs=2) as mid, \
             tc.tile_pool(name="psG", bufs=2, space="PSUM") as psG, \
             tc.tile_pool(name="psPR", bufs=2, space="PSUM") as psPR, \
             tc.tile_pool(name="psLG", bufs=1, space="PSUM") as psLG, \
             tc.tile_pool(name="psQT", bufs=2, space="PSUM") as psQT, \
             tc.tile_pool(name="psT", bufs=1, space="PSUM") as psT:

            # ---- constants (loaded once) ----
            nm2b_t = const.tile([128, NBLK], f32)
            nc.sync.dma_start(out=nm2b_t, in_=nm2b[:, :])
            sel8_t = const.tile([2 * BLK, 128], bf16)
            nc.sync.dma_start(out=sel8_t, in_=sel8[:, :])
            sel80_t = const.tile([BLK * W, 128], bf16)
            nc.sync.dma_start(out=sel80_t, in_=sel80[:, :])
            ident = const.tile([128, 128], bf16)
            make_identity(nc, ident)
            # PE warmup: dense dummy matmuls during the DMA ramp so the HAM
            # un-throttles (1.2 -> 2.4 GHz) before the first real block.
            wups = psLG.tile([128, 128], f32, tag="lgT")
            for _ in range(80):
                nc.tensor.matmul(wups, ident, ident, start=True, stop=True)
            wupd = const.tile([128, 128], bf16)
            nc.vector.tensor_copy(wupd, wups)

            def s_dma(b, st):
                """input DMAs for block b."""
                qTt = inp.tile([128, DC, BLK, Q], bf16, tag="qTt", bufs=4)
                nc.sync.dma_start(
                    out=qTt,
                    in_=qT[:, b, :].rearrange("p (c j q) -> p c j q", c=DC, j=BLK),
                )
                mTs_t = inp.tile([128, DC, BLK, W], bf16, tag="mTs", bufs=3)
                nc.sync.dma_start(
                    out=mTs_t,
                    in_=mTs[:, b, :].rearrange("p (c j w) -> p c j w", c=DC, j=BLK),
                )
                ssum_t = inp.tile([BLK * W, D], bf16, tag="ssum", bufs=3)
                nc.sync.dma_start(out=ssum_t, in_=ssum_p[:, b, :])
                nq2_t = inp.tile([2 * BLK, Q], bf16, tag="nq2", bufs=3)
                nc.sync.dma_start(out=nq2_t, in_=nq2hl[:, b, :])
                st.update(qTt=qTt, mTs_t=mTs_t, ssum_t=ssum_t, nq2_t=nq2_t)

            def s_g(b, st):
                """G psum = nq2 (K=8 selector) - 2*mean.T @ q, col-tiled."""
                qTt, mTs_t, nq2_t = st["qTt"], st["mTs_t"], st["nq2_t"]
                g = psG.tile([128, Q], f32, tag="g")
                # emit in waves (all 4 col-groups adjacent) so the PE streams
                # the four 32-col chains concurrently
                for j in range(BLK):
                    nc.tensor.matmul(
                        g[32 * j:32 * j + 32, :],
                        sel8_t[:, 32 * j:32 * j + 32], nq2_t,
                        start=True, stop=False,
                        tile_position=(0, 32 * j),
                    )
                for c in range(DC):
                    for j in range(BLK):
                        nc.tensor.matmul(
                            g[32 * j:32 * j + W, :],
                            mTs_t[:, c, j, :], qTt[:, c, j, :],
                            start=False, stop=(c == DC - 1),
                            tile_position=(0, 32 * j),
                        )
                st["g"] = g

            def s_softmax(b, st):
                """dist chain from G psum; w37 bf16 [128, Q]."""
                g = st["g"]
                lnv = mid.tile([128, Q], f32, tag="lnv")
                nc.scalar.activation(out=lnv, in_=g,
                                     func=mybir.ActivationFunctionType.Ln,
                                     bias=nm2b_t[:, b:b + 1], scale=1.0)
                dist = mid.tile([128, Q], f32, tag="dist")
                nc.scalar.activation(out=dist, in_=lnv,
                                     func=mybir.ActivationFunctionType.Exp,
                                     bias=0.0, scale=0.5)
                wexp = mid.tile([128, Q], f32, tag="wexp")
                sums = mid.tile([128, 1], f32, tag="sums")
                nc.scalar.activation(out=wexp, in_=dist,
                                     func=mybir.ActivationFunctionType.Exp,
                                     bias=0.0, scale=-1.0, accum_out=sums)
                recip = mid.tile([128, 1], f32, tag="recip")
                nc.vector.reciprocal(recip, sums)
                w37 = mid.tile([128, Q], bf16, tag="w37")
                nc.vector.tensor_scalar(
                    out=w37, in0=wexp, scalar1=recip, scalar2=ALPHA,
                    op0=mybir.AluOpType.mult, op1=mybir.AluOpType.mult,
                )
                st["w37"] = w37

            def s_qtrans(b, st):
                """on-chip transpose qT -> qnat chunks for the pooled matmul."""
                qTt = st["qTt"]
                qn_t = []
                for ci, (q0, cnt) in enumerate(QCH):
                    t = qnp.tile([128, BLK, D], bf16, tag=f"qn{ci}")
                    qn_t.append(t)
                st["qn_t"] = qn_t
                k = 0
                for j in range(BLK):
                    for ci, (q0, cnt) in enumerate(QCH):
                        tp = psQT.tile([128, DC, 128], bf16, tag="qtp")
                        for c in range(DC):
                            nc.tensor.transpose(
                                tp[0:cnt, c, :], qTt[:, c, j, q0:q0 + cnt], ident)
                        tpf = tp[0:cnt, :, :].rearrange("p c d -> p (c d)")
                        # alternate copy engine to split the psum->sbuf load
                        if k % 2 == 0:
                            nc.scalar.copy(qn_t[ci][0:cnt, j, :], tpf)
                        else:
                            nc.vector.tensor_copy(qn_t[ci][0:cnt, j, :], tpf)
                        k += 1

            def s_wt(b, st):
                """wT transposes + copies."""
                w37 = st["w37"]
                wTps = psT.tile([128, len(QCH), 128], bf16, tag="tp")
                for ci, (q0, cnt) in enumerate(QCH):
                    nc.tensor.transpose(wTps[0:cnt, ci, :], w37[:, q0:q0 + cnt], ident)
                wTsb = []
                for ci, (q0, cnt) in enumerate(QCH):
                    t = mid.tile([128, 128], bf16, tag=f"wTsb{ci}")
                    nc.vector.tensor_copy(t[0:cnt], wTps[0:cnt, ci, :])
                    wTsb.append(t)
                st["wTsb"] = wTsb

            def s_m2(b, st):
                """proto accumulation: ssum (K=80 selector) + 37*pooled; l2norm."""
                wTsb, qn_t, ssum_t = st["wTsb"], st["qn_t"], st["ssum_t"]
                pr = psPR.tile([128, D], f32, tag="pr")
                for j in range(BLK):
                    nc.tensor.matmul(
                        pr[32 * j:32 * j + 32, :],
                        sel80_t[:, 32 * j:32 * j + 32], ssum_t,
                        start=True, stop=False,
                        tile_position=(0, 32 * j),
                    )
                for ci, (q0, cnt) in enumerate(QCH):
                    for j in range(BLK):
                        nc.tensor.matmul(
                            pr[32 * j:32 * j + W, :],
                            wTsb[ci][0:cnt, 32 * j:32 * j + W],
                            qn_t[ci][0:cnt, j, :],
                            start=False, stop=(ci == len(QCH) - 1),
                            tile_position=(0, 32 * j),
                        )
                sqdump = mid.tile([128, D], bf16, tag="sqdump")
                n2 = mid.tile([128, 1], f32, tag="n2")
                nc.scalar.activation(out=sqdump, in_=pr,
                                     func=mybir.ActivationFunctionType.Square,
                                     bias=0.0, scale=1.0, accum_out=n2)
                lnn = mid.tile([128, 1], f32, tag="lnn")
                nc.scalar.activation(out=lnn, in_=n2,
                                     func=mybir.ActivationFunctionType.Ln,
                                     bias=0.0, scale=1.0)
                rstd = mid.tile([128, 1], f32, tag="rstd")
                nc.scalar.activation(out=rstd, in_=lnn,
                                     func=mybir.ActivationFunctionType.Exp,
                                     bias=0.0, scale=-0.5)
                proton = mid.tile([128, D], bf16, tag="proton")
                nc.vector.tensor_scalar_mul(out=proton, in0=pr, scalar1=rstd)
                st["proton"] = proton

            def s_pt(b, st):
                """protonT transposes + copies."""
                ptps = psT.tile([128, DC, 128], bf16, tag="tp")
                for c in range(DC):
                    nc.tensor.transpose(ptps[:, c, :], st["proton"][:, 128 * c:128 * (c + 1)], ident)
                ptsb = []
                for c in range(DC):
                    t = mid.tile([128, 128], bf16, tag=f"ptsb{c}")
                    nc.vector.tensor_copy(t, ptps[:, c, :])
                    ptsb.append(t)
                st["ptsb"] = ptsb

            def s_m3(b, st):
                """logits matmul (transposed layout)."""
                ptsb, qTt = st["ptsb"], st["qTt"]
                lgT = psLG.tile([128, Q], f32, tag="lgT")
                for c in range(DC):
                    for j in range(BLK):
                        nc.tensor.matmul(
                            lgT[32 * j:32 * j + W, :],
                            ptsb[c][:, 32 * j:32 * j + W],
                            qTt[:, c, j, :],
                            start=(c == 0), stop=(c == DC - 1),
                            tile_position=(0, 32 * j),
                        )
                st["lgT"] = lgT

            def s_out(b, st):
                """store raw transposed logits; host does final transpose+scale."""
                lgsb = mid.tile([128, Q], bf16, tag="lgsb")
                nc.scalar.copy(lgsb, st["lgT"])
                nc.gpsimd.dma_start(out=outT[:, b, :], in_=lgsb)

            # 4-deep software pipeline; PE never waits on ACT/DVE chains.
            # PE order/iter t: G_t | wT_{t-1} | qnatT_t | M3_{t-3} | pT_{t-2} | M2_{t-1}
            sts = {}
            for t in range(NBLK + 3):
                if t < NBLK:
                    sts[t] = {}
                    s_dma(t, sts[t])
                    s_g(t, sts[t])
                if 0 <= t - 1 < NBLK:
                    s_wt(t - 1, sts[t - 1])
                if t < NBLK:
                    s_softmax(t, sts[t])
                    s_qtrans(t, sts[t])
                if 0 <= t - 3 < NBLK:
                    s_m3(t - 3, sts[t - 3])
                    s_out(t - 3, sts[t - 3])
                if 0 <= t - 2 < NBLK:
                    s_pt(t - 2, sts[t - 2])
                if 0 <= t - 1 < NBLK:
                    s_m2(t - 1, sts[t - 1])
                if 0 <= t - 4 < NBLK:
                    del sts[t - 4]

    nc.finalize()
    return nc


def _get_built():
    global _BUILT
    if _BUILT is None:
        _BUILT = _build()
    return _BUILT


def _prep_core_inputs(x_shot, x_query, temp):
    """x_shot [EL,W,S,D] f32, x_query [EL,Q,D] f32 -> input map for one core."""
    qTr = x_query.transpose(2, 0, 1).reshape(DC, 128, NBLK, BLK, Q)
    qTr = np.ascontiguousarray(qTr.transpose(1, 2, 0, 3, 4)).reshape(128, NBLK, DC * BLK * Q)
    qTr = qTr.astype(ml_dtypes.bfloat16)

    shot_sum = x_shot.sum(axis=2)                    # [EL, W, D] f32
    mean = shot_sum / S
    # ssum packed rows: k = 20*j + w  ->  episode BLK*b+j
    ssp = shot_sum.reshape(NBLK, BLK * W, D).transpose(1, 0, 2)
    ssp = np.ascontiguousarray(ssp).astype(ml_dtypes.bfloat16)
    # mTs[p, b, (c j w)] = -2 * mean[4b+j, w, 128c+p]
    m = (-2.0 * mean).reshape(NBLK, BLK, W, DC, 128)
    m = m.transpose(4, 0, 3, 1, 2).reshape(128, NBLK, DC * BLK * W)
    mTs = np.ascontiguousarray(m).astype(ml_dtypes.bfloat16)

    nq2 = np.einsum("eqd,eqd->eq", x_query.astype(np.float64),
                    x_query.astype(np.float64)).astype(np.float32)   # [EL, Q]
    nq2b = nq2.reshape(NBLK, BLK, Q)                 # [b, j, q]
    hi = nq2b.astype(ml_dtypes.bfloat16)
    lo = (nq2b - hi.astype(np.float32)).astype(ml_dtypes.bfloat16)
    nq2hl = np.concatenate([hi, lo], axis=1)         # [b, 2*BLK, q]
    nq2hl = np.ascontiguousarray(nq2hl.transpose(1, 0, 2))  # [2*BLK, b, q]

    nm2 = np.einsum("ewd,ewd->ew", mean, mean)       # [EL, W] f32
    nm2b = np.zeros((128, NBLK), np.float32)
    for b in range(NBLK):
        for j in range(BLK):
            nm2b[32 * j:32 * j + W, b] = nm2[BLK * b + j]

    return {
        "qT": qTr, "mTs": mTs, "ssum_p": ssp, "nq2hl": nq2hl, "nm2b": nm2b,
    }


def _consts():
    sel8 = np.zeros((2 * BLK, 128), np.float32)
    for j in range(BLK):
        sel8[j, 32 * j:32 * j + W] = 1.0       # hi rows
        sel8[BLK + j, 32 * j:32 * j + W] = 1.0  # lo rows
    sel80 = np.zeros((BLK * W, 128), np.float32)
    for j in range(BLK):
        for w in range(W):
            sel80[W * j + w, 32 * j + w] = 1.0
    return {
        "sel8": sel8.astype(ml_dtypes.bfloat16),
        "sel80": sel80.astype(ml_dtypes.bfloat16),
    }


def _run(x_shot, x_query, temp, trace=False):
    nc = _get_built()
    consts = _consts()
    in_maps = []
    for i in range(NCORES):
        sl = slice(i * EL, (i + 1) * EL)
        m = _prep_core_inputs(x_shot[sl], x_query[sl], temp)
        m.update(consts)
        in_maps.append(m)
    res = run_bass_kernel_spmd(
        nc, in_maps, list(range(NCORES)), trace=trace,
        tmpdir="/tmp/bass_trace_out" if trace else None,
    )
    out = np.empty((E, Q, W), np.float32)
    for i in range(NCORES):
        sl = slice(i * EL, (i + 1) * EL)
        nq2 = np.einsum("eqd,eqd->eq", x_query[sl].astype(np.float64),
                        x_query[sl].astype(np.float64)).astype(np.float32)
        qscale = (np.float32(temp) / np.sqrt(nq2))[:, :, None]   # [EL, Q, 1]
        raw = res.results[i]["outT"].astype(np.float32).reshape(4, 32, NBLK, Q)
        lg = raw[:, 0:W].transpose(2, 0, 1, 3).reshape(EL, W, Q) # [e, w, q]
        out[sl] = lg.transpose(0, 2, 1) * qscale
    return out, res


def kernel(x_shot, x_query, temp):
    x_shot = np.asarray(x_shot, dtype=np.float32)
    x_query = np.asarray(x_query, dtype=np.float32)
    out, _ = _run(x_shot, x_query, np.float32(temp))
    return out


def kernel_timed(x_shot, x_query, temp):
    x_shot = np.asarray(x_shot, dtype=np.float32)
    x_query = np.asarray(x_query, dtype=np.float32)
    out, res = _run(x_shot, x_query, np.float32(temp), trace=True)
    return out, res


# revision 17
# speedup vs baseline: 1.3427x; 1.0101x over previous
"""MetaBaseline (retrieval_knn) Trainium2 kernel.

Problem: E=256 episodes; per episode:
  shot_sum[W,D], shot_mean = mean over S shots
  dist[W,Q]   = ||shot_mean_w - q_q||_2
  weights     = softmax(-dist, axis=Q)
  pooled[W,D] = weights @ x_query
  proto       = l2norm(shot_sum + 37*pooled)       (the /42 cancels in l2norm)
  logits[Q,W] = temp * l2norm(x_query) @ proto.T

Sharding: pure data parallel over E across 8 NeuronCores (32 episodes/core).
On-device layout: blocks of 4 episodes packed on the partition dim at
32-partition offsets (col-tiled matmuls), so softmax/activation work runs on
[128, Q] tiles serving 4 episodes at once.

v2 design (vs v1): x_query ships ONCE (transposed layout qT only, 9.8MB/core);
the natural-layout copy needed by the pooled matmul is produced on-chip with
PE transposes (48 [128,<=128] transposes/block) + ACT/DVE psum->sbuf copies.
nq2 (per-query norms) broadcast into the dist psum via a K=8 selector matmul
(bf16 hi+lo rows for f32-grade accuracy); shot_sum added into the proto psum
via a K=80 selector matmul of packed rows. Software pipeline uses a 4-deep
skew so the PE never waits on the serial ACT softmax/l2norm chains:

  iter t PE order: G_t | qnatT_t (48 tp) | M3_{t-3} | wT_{t-1} | pT_{t-2} | M2_{t-1}
  ACT during t:    softmax_t (Ln/Exp/Exp+accum), l2norm_{t-1}, lgsb_{t-3}, half qn copies
  DVE during t:    recip_t, w37_t, proton_{t-1}, wT/pT copies, half qn copies
  GPSIMD:          outT DMA (t-3)

Per-core DMA ~12.9MB (36us floor at 358GB/s); PE ~5us/block warm.
Host does cheap O(N*D) prep (sums/norms/layout packs) and the final
transpose + temp/||q|| scaling of the raw logits.
"""
import sys

sys.path.insert(0, "/opt/trn_rl_repo")

import numpy as np
import ml_dtypes

import concourse.bass as bass
import concourse.tile as tile
from concourse import bacc, mybir
from concourse.bass_utils import run_bass_kernel_spmd
from concourse.masks import make_identity

bf16 = mybir.dt.bfloat16
f32 = mybir.dt.float32

E, W, S, Q, D = 256, 20, 5, 300, 512
ALPHA = 37.0
NCORES = 8
EL = E // NCORES      # 32 episodes per core
BLK = 4               # episodes per block (packed at 32-partition offsets)
NBLK = EL // BLK      # 8 blocks
DC = D // 128         # 4 K-chunks over D
QCH = [(0, 128), (128, 128), (256, Q - 256)]  # q chunks (offset, count)

_BUILT = None


def _pin_act_table_set():
    """Make Bacc's ACT-table-load pass pick one covering set for Ln/Exp/Square.

    The pass walks activations and loads the first set containing the needed
    function; Ln's first set lacks Exp and vice versa, so alternating
    Ln/Exp/Square thrashes ACT_TABLE_LOAD (~1.3us each). Hide those functions
    from every set except natural_log_exp_and_others (set *indices* are
    preserved — contents of the real act_info.json are untouched).
    """
    import concourse.bacc as bacc_mod
    from concourse import hw_specs

    if getattr(bacc_mod, "_act_tables_pinned", False):
        return
    orig = hw_specs.get_activation_tables
    pin = {
        mybir.ActivationFunctionType.Ln,
        mybir.ActivationFunctionType.Exp,
        mybir.ActivationFunctionType.Square,
    }
    keep = "natural_log_exp_and_others"

    def pinned(arch):
        tabs = orig(arch)
        return {
            name: set(fns) if name == keep else (set(fns) - pin)
            for name, fns in tabs.items()
        }

    bacc_mod.get_activation_tables = pinned
    bacc_mod._act_tables_pinned = True


def _build():
    _pin_act_table_set()
    nc = bacc.Bacc("TRN2", target_bir_lowering=False, debug=False)

    qT = nc.declare_dram_parameter("qT", [128, NBLK, DC * BLK * Q], bf16, isOutput=False)
    mTs = nc.declare_dram_parameter("mTs", [128, NBLK, DC * BLK * W], bf16, isOutput=False)
    ssum_p = nc.declare_dram_parameter("ssum_p", [BLK * W, NBLK, D], bf16, isOutput=False)
    nq2hl = nc.declare_dram_parameter("nq2hl", [2 * BLK, NBLK, Q], bf16, isOutput=False)
    nm2b = nc.declare_dram_parameter("nm2b", [128, NBLK], f32, isOutput=False)
    sel8 = nc.declare_dram_parameter("sel8", [2 * BLK, 128], bf16, isOutput=False)
    sel80 = nc.declare_dram_parameter("sel80", [BLK * W, 128], bf16, isOutput=False)
    outT = nc.declare_dram_parameter("outT", [128, NBLK, Q], bf16, isOutput=True)

    with tile.TileContext(nc) as tc:
        with tc.tile_pool(name="const", bufs=1) as const, \
             tc.tile_pool(name="inp", bufs=2) as inp, \
             tc.tile_pool(name="qn", bufs=2) as qnp, \
             tc.tile_pool(name="mid", bufs=2) as mid, \
             tc.tile_pool(name="psG", bufs=2, space="PSUM") as psG, \
             tc.tile_pool(name="psPR", bufs=2, space="PSUM") as psPR, \
             tc.tile_pool(name="psLG", bufs=1, space="PSUM") as psLG, \
             tc.tile_pool(name="psQT", bufs=2, space="PSUM") as psQT, \
             tc.tile_pool(name="psT", bufs=1, space="PSUM") as psT:
            # PSUM banks: g x2 + pr x2 + lgT x1 + qtpA x2 + tp x1 = 8

            # ---- constants (loaded once) ----
            nm2b_t = const.tile([128, NBLK], f32)
            nc.sync.dma_start(out=nm2b_t, in_=nm2b[:, :])
            sel8_t = const.tile([2 * BLK, 128], bf16)
            nc.sync.dma_start(out=sel8_t, in_=sel8[:, :])
            sel80_t = const.tile([BLK * W, 128], bf16)
            nc.sync.dma_start(out=sel80_t, in_=sel80[:, :])
            ident = const.tile([128, 128], bf16)
            make_identity(nc, ident)
            # PE warmup: dense dummy matmuls during the DMA ramp so the HAM
            # un-throttles (1.2 -> 2.4 GHz) before the first real block.
            wups = psLG.tile([128, 128], f32, tag="lgT")
            for _ in range(80):
                nc.tensor.matmul(wups, ident, ident, start=True, stop=True)
            wupd = const.tile([128, 128], bf16)
            nc.vector.tensor_copy(wupd, wups)

            def s_dma(b, st):
                """input DMAs for block b."""
                qTt = inp.tile([128, DC, BLK, Q], bf16, tag="qTt", bufs=4)
                nc.sync.dma_start(
                    out=qTt,
                    in_=qT[:, b, :].rearrange("p (c j q) -> p c j q", c=DC, j=BLK),
                )
                mTs_t = inp.tile([128, DC, BLK, W], bf16, tag="mTs", bufs=3)
                nc.sync.dma_start(
                    out=mTs_t,
                    in_=mTs[:, b, :].rearrange("p (c j w) -> p c j w", c=DC, j=BLK),
                )
                ssum_t = inp.tile([BLK * W, D], bf16, tag="ssum", bufs=3)
                nc.sync.dma_start(out=ssum_t, in_=ssum_p[:, b, :])
                nq2_t = inp.tile([2 * BLK, Q], bf16, tag="nq2", bufs=3)
                nc.sync.dma_start(out=nq2_t, in_=nq2hl[:, b, :])
                st.update(qTt=qTt, mTs_t=mTs_t, ssum_t=ssum_t, nq2_t=nq2_t)

            def s_g(b, st):
                """G psum = nq2 (K=8 selector) - 2*mean.T @ q, col-tiled."""
                qTt, mTs_t, nq2_t = st["qTt"], st["mTs_t"], st["nq2_t"]
                g = psG.tile([128, Q], f32, tag="g")
                # emit in waves (all 4 col-groups adjacent) so the PE streams
                # the four 32-col chains concurrently
                for j in range(BLK):
                    nc.tensor.matmul(
                        g[32 * j:32 * j + 32, :],
                        sel8_t[:, 32 * j:32 * j + 32], nq2_t,
                        start=True, stop=False,
                        tile_position=(0, 32 * j),
                    )
                for c in range(DC):
                    for j in range(BLK):
                        nc.tensor.matmul(
                            g[32 * j:32 * j + W, :],
                            mTs_t[:, c, j, :], qTt[:, c, j, :],
                            start=False, stop=(c == DC - 1),
                            tile_position=(0, 32 * j),
                        )
                st["g"] = g

            def s_softmax_act(b, st):
                """dist chain from G psum (ACT half)."""
                g = st["g"]
                lnv = mid.tile([128, Q], f32, tag="lnv")
                nc.scalar.activation(out=lnv, in_=g,
                                     func=mybir.ActivationFunctionType.Ln,
                                     bias=nm2b_t[:, b:b + 1], scale=1.0)
                dist = mid.tile([128, Q], f32, tag="dist")
                nc.scalar.activation(out=dist, in_=lnv,
                                     func=mybir.ActivationFunctionType.Exp,
                                     bias=0.0, scale=0.5)
                wexp = mid.tile([128, Q], f32, tag="wexp")
                sums = mid.tile([128, 1], f32, tag="sums")
                nc.scalar.activation(out=wexp, in_=dist,
                                     func=mybir.ActivationFunctionType.Exp,
                                     bias=0.0, scale=-1.0, accum_out=sums)
                st.update(wexp=wexp, sums=sums)

            def s_softmax_dve(b, st):
                """softmax normalization (DVE half): w37 = wexp * (37/sums)."""
                recip = mid.tile([128, 1], f32, tag="recip")
                nc.vector.reciprocal(recip, st["sums"])
                w37 = mid.tile([128, Q], bf16, tag="w37")
                nc.vector.tensor_scalar(
                    out=w37, in0=st["wexp"], scalar1=recip, scalar2=ALPHA,
                    op0=mybir.AluOpType.mult, op1=mybir.AluOpType.mult,
                )
                st["w37"] = w37

            def s_qn_alloc(b, st):
                qn_t = qnp.tile([128, len(QCH), BLK, D], bf16, tag="qn")
                st["qn_t"] = qn_t

            def s_qtrans_j(b, st, j):
                """on-chip transpose of episode j's qT -> qn[:, :, j, :]."""
                qTt, qn_t = st["qTt"], st["qn_t"]
                # chunks 0,1 (cnt=128): 8 transposes -> one merged 2KB-psum copy
                tpA = psQT.tile([128, 2, DC, 128], bf16, tag="qtpA")
                for a in range(2):
                    q0, cnt = QCH[a]
                    for c in range(DC):
                        nc.tensor.transpose(
                            tpA[:, a, c, :], qTt[:, c, j, q0:q0 + cnt], ident)
                srcA = tpA.rearrange("p a c d -> p a (c d)")
                dstA = qn_t[:, 0:2, j, :]
                if j != 2:
                    nc.vector.tensor_copy(dstA, srcA)
                else:
                    nc.scalar.copy(dstA, srcA)
                # chunk 2 (cnt=44): shares the 'tp' psum bank
                q0, cnt = QCH[2]
                tpB = psT.tile([128, DC, 128], bf16, tag="tp")
                for c in range(DC):
                    nc.tensor.transpose(
                        tpB[0:cnt, c, :], qTt[:, c, j, q0:q0 + cnt], ident)
                srcB = tpB[0:cnt, :, :].rearrange("p c d -> p (c d)")
                dstB = qn_t[0:cnt, 2, j, :]
                if j < 2:
                    nc.scalar.copy(dstB, srcB)
                else:
                    nc.vector.tensor_copy(dstB, srcB)

            def s_wt(b, st):
                """wT transposes + merged copies."""
                w37 = st["w37"]
                wTps = psT.tile([128, len(QCH), 128], bf16, tag="tp")
                for ci, (q0, cnt) in enumerate(QCH):
                    nc.tensor.transpose(wTps[0:cnt, ci, :], w37[:, q0:q0 + cnt], ident)
                wTsb = mid.tile([128, len(QCH), 128], bf16, tag="wTsb")
                nc.vector.tensor_copy(wTsb[:, 0:2, :], wTps[:, 0:2, :])
                nc.vector.tensor_copy(wTsb[0:QCH[2][1], 2, :], wTps[0:QCH[2][1], 2, :])
                st["wTsb"] = wTsb

            def s_m2(b, st):
                """proto psum: ssum (K=80 selector) + 37*pooled; n2 via DVE."""
                wTsb, qn_t, ssum_t = st["wTsb"], st["qn_t"], st["ssum_t"]
                pr = psPR.tile([128, D], f32, tag="pr")
                for j in range(BLK):
                    nc.tensor.matmul(
                        pr[32 * j:32 * j + 32, :],
                        sel80_t[:, 32 * j:32 * j + 32], ssum_t,
                        start=True, stop=False,
                        tile_position=(0, 32 * j),
                    )
                for ci, (q0, cnt) in enumerate(QCH):
                    for j in range(BLK):
                        nc.tensor.matmul(
                            pr[32 * j:32 * j + W, :],
                            wTsb[0:cnt, ci, 32 * j:32 * j + W],
                            qn_t[0:cnt, ci, j, :],
                            start=False, stop=(ci == len(QCH) - 1),
                            tile_position=(0, 32 * j),
                        )
                # unnormalized proton: the 1/||proto|| lands on the logits rows
                proton = mid.tile([128, D], bf16, tag="proton")
                nc.vector.tensor_copy(proton, pr)
                sqdump = mid.tile([128, D], bf16, tag="sqdump")
                n2 = mid.tile([128, 1], f32, tag="n2")
                nc.scalar.activation(out=sqdump, in_=proton,
                                     func=mybir.ActivationFunctionType.Square,
                                     bias=0.0, scale=1.0, accum_out=n2)
                lnn = mid.tile([128, 1], f32, tag="lnn")
                nc.scalar.activation(out=lnn, in_=n2,
                                     func=mybir.ActivationFunctionType.Ln,
                                     bias=0.0, scale=1.0)
                rstd = mid.tile([128, 1], f32, tag="rstd")
                nc.scalar.activation(out=rstd, in_=lnn,
                                     func=mybir.ActivationFunctionType.Exp,
                                     bias=0.0, scale=-0.5)
                st.update(proton=proton, rstd=rstd)

            def s_pt(b, st):
                """protonT transposes + one merged copy."""
                ptps = psT.tile([128, DC, 128], bf16, tag="tp")
                for c in range(DC):
                    nc.tensor.transpose(ptps[:, c, :], st["proton"][:, 128 * c:128 * (c + 1)], ident)
                ptsb = mid.tile([128, DC, 128], bf16, tag="ptsb")
                nc.vector.tensor_copy(ptsb, ptps)
                st["ptsb"] = ptsb

            def s_m3(b, st):
                """logits matmul (transposed layout)."""
                ptsb, qTt = st["ptsb"], st["qTt"]
                lgT = psLG.tile([128, Q], f32, tag="lgT")
                for c in range(DC):
                    for j in range(BLK):
                        nc.tensor.matmul(
                            lgT[32 * j:32 * j + W, :],
                            ptsb[:, c, 32 * j:32 * j + W],
                            qTt[:, c, j, :],
                            start=(c == 0), stop=(c == DC - 1),
                            tile_position=(0, 32 * j),
                        )
                st["lgT"] = lgT

            def s_out(b, st):
                """scale rows by 1/||proto|| while copying psum->sbuf; DMA out."""
                lgsb = mid.tile([128, Q], bf16, tag="lgsb")
                nc.vector.tensor_scalar_mul(out=lgsb, in0=st["lgT"], scalar1=st["rstd"])
                nc.gpsimd.dma_start(out=outT[:, b, :], in_=lgsb)

            # 4-deep software pipeline. qtrans groups interleave with the
            # matmul waves so the HAM activity monitor (which ignores
            # transpose-mode work) keeps seeing regular matmuls.
            # PE/iter t: wT_{t-1} | G_t | qj0_t | M2_{t-1} | qj1_t | pT_{t-2}
            #            | qj2_t | M3_{t-3} | qj3_t
            sts = {}
            for t in range(NBLK + 3):
                if 0 <= t - 1 < NBLK:
                    s_wt(t - 1, sts[t - 1])
                if t < NBLK:
                    sts[t] = {}
                    s_dma(t, sts[t])
                    s_g(t, sts[t])
                    s_softmax_act(t, sts[t])
                    s_qn_alloc(t, sts[t])
                    s_qtrans_j(t, sts[t], 0)
                    s_softmax_dve(t, sts[t])
                if 0 <= t - 1 < NBLK:
                    s_m2(t - 1, sts[t - 1])
                if t < NBLK:
                    s_qtrans_j(t, sts[t], 1)
                if 0 <= t - 2 < NBLK:
                    s_pt(t - 2, sts[t - 2])
                if t < NBLK:
                    s_qtrans_j(t, sts[t], 2)
                if 0 <= t - 3 < NBLK:
                    s_m3(t - 3, sts[t - 3])
                    s_out(t - 3, sts[t - 3])
                if t < NBLK:
                    s_qtrans_j(t, sts[t], 3)
                if 0 <= t - 4 < NBLK:
                    del sts[t - 4]

    nc.finalize()
    return nc


def _get_built():
    global _BUILT
    if _BUILT is None:
        _BUILT = _build()
    return _BUILT


def _prep_core_inputs(x_shot, x_query, temp):
    """x_shot [EL,W,S,D] f32, x_query [EL,Q,D] f32 -> input map for one core."""
    qTr = x_query.transpose(2, 0, 1).reshape(DC, 128, NBLK, BLK, Q)
    qTr = np.ascontiguousarray(qTr.transpose(1, 2, 0, 3, 4)).reshape(128, NBLK, DC * BLK * Q)
    qTr = qTr.astype(ml_dtypes.bfloat16)

    shot_sum = x_shot.sum(axis=2)                    # [EL, W, D] f32
    mean = shot_sum / S
    # ssum packed rows: k = 20*j + w  ->  episode BLK*b+j
    ssp = shot_sum.reshape(NBLK, BLK * W, D).transpose(1, 0, 2)
    ssp = np.ascontiguousarray(ssp).astype(ml_dtypes.bfloat16)
    # mTs[p, b, (c j w)] = -2 * mean[4b+j, w, 128c+p]
    m = (-2.0 * mean).reshape(NBLK, BLK, W, DC, 128)
    m = m.transpose(4, 0, 3, 1, 2).reshape(128, NBLK, DC * BLK * W)
    mTs = np.ascontiguousarray(m).astype(ml_dtypes.bfloat16)

    nq2 = np.einsum("eqd,eqd->eq", x_query.astype(np.float64),
                    x_query.astype(np.float64)).astype(np.float32)   # [EL, Q]
    nq2b = nq2.reshape(NBLK, BLK, Q)                 # [b, j, q]
    hi = nq2b.astype(ml_dtypes.bfloat16)
    lo = (nq2b - hi.astype(np.float32)).astype(ml_dtypes.bfloat16)
    nq2hl = np.concatenate([hi, lo], axis=1)         # [b, 2*BLK, q]
    nq2hl = np.ascontiguousarray(nq2hl.transpose(1, 0, 2))  # [2*BLK, b, q]

    nm2 = np.einsum("ewd,ewd->ew", mean, mean)       # [EL, W] f32
    nm2b = np.zeros((128, NBLK), np.float32)
    for b in range(NBLK):
        for j in range(BLK):
            nm2b[32 * j:32 * j + W, b] = nm2[BLK * b + j]

    return {
        "qT": qTr, "mTs": mTs, "ssum_p": ssp, "nq2hl": nq2hl, "nm2b": nm2b,
    }


def _consts():
    sel8 = np.zeros((2 * BLK, 128), np.float32)
    for j in range(BLK):
        sel8[j, 32 * j:32 * j + W] = 1.0       # hi rows
        sel8[BLK + j, 32 * j:32 * j + W] = 1.0  # lo rows
    sel80 = np.zeros((BLK * W, 128), np.float32)
    for j in range(BLK):
        for w in range(W):
            sel80[W * j + w, 32 * j + w] = 1.0
    return {
        "sel8": sel8.astype(ml_dtypes.bfloat16),
        "sel80": sel80.astype(ml_dtypes.bfloat16),
    }


def _run(x_shot, x_query, temp, trace=False):
    nc = _get_built()
    consts = _consts()
    in_maps = []
    for i in range(NCORES):
        sl = slice(i * EL, (i + 1) * EL)
        m = _prep_core_inputs(x_shot[sl], x_query[sl], temp)
        m.update(consts)
        in_maps.append(m)
    res = run_bass_kernel_spmd(
        nc, in_maps, list(range(NCORES)), trace=trace,
        tmpdir="/tmp/bass_trace_out" if trace else None,
    )
    out = np.empty((E, Q, W), np.float32)
    for i in range(NCORES):
        sl = slice(i * EL, (i + 1) * EL)
        nq2 = np.einsum("eqd,eqd->eq", x_query[sl].astype(np.float64),
                        x_query[sl].astype(np.float64)).astype(np.float32)
        qscale = (np.float32(temp) / np.sqrt(nq2))[:, :, None]   # [EL, Q, 1]
        raw = res.results[i]["outT"].astype(np.float32).reshape(4, 32, NBLK, Q)
        lg = raw[:, 0:W].transpose(2, 0, 1, 3).reshape(EL, W, Q) # [e, w, q]
        out[sl] = lg.transpose(0, 2, 1) * qscale
    return out, res


def kernel(x_shot, x_query, temp):
    x_shot = np.asarray(x_shot, dtype=np.float32)
    x_query = np.asarray(x_query, dtype=np.float32)
    out, _ = _run(x_shot, x_query, np.float32(temp))
    return out


def kernel_timed(x_shot, x_query, temp):
    x_shot = np.asarray(x_shot, dtype=np.float32)
    x_query = np.asarray(x_query, dtype=np.float32)
    out, res = _run(x_shot, x_query, np.float32(temp), trace=True)
    return out, res


# revision 18
# speedup vs baseline: 1.4121x; 1.0516x over previous
"""MetaBaseline (retrieval_knn) Trainium2 kernel.

Problem: E=256 episodes; per episode:
  shot_sum[W,D], shot_mean = mean over S shots
  dist[W,Q]   = ||shot_mean_w - q_q||_2
  weights     = softmax(-dist, axis=Q)
  pooled[W,D] = weights @ x_query
  proto       = l2norm(shot_sum + 37*pooled)       (the /42 cancels in l2norm)
  logits[Q,W] = temp * l2norm(x_query) @ proto.T

Sharding: pure data parallel over E across 8 NeuronCores (32 episodes/core).
On-device layout: blocks of 4 episodes packed on the partition dim at
32-partition offsets (col-tiled matmuls), so softmax/activation work runs on
[128, Q] tiles serving 4 episodes at once.

v2 design (vs v1): x_query ships ONCE (transposed layout qT only, 9.8MB/core);
the natural-layout copy needed by the pooled matmul is produced on-chip with
PE transposes (48 [128,<=128] transposes/block) + ACT/DVE psum->sbuf copies.
nq2 (per-query norms) broadcast into the dist psum via a K=8 selector matmul
(bf16 hi+lo rows for f32-grade accuracy); shot_sum added into the proto psum
via a K=80 selector matmul of packed rows. Software pipeline uses a 4-deep
skew so the PE never waits on the serial ACT softmax/l2norm chains:

  iter t PE order: G_t | qnatT_t (48 tp) | M3_{t-3} | wT_{t-1} | pT_{t-2} | M2_{t-1}
  ACT during t:    softmax_t (Ln/Exp/Exp+accum), l2norm_{t-1}, lgsb_{t-3}, half qn copies
  DVE during t:    recip_t, w37_t, proton_{t-1}, wT/pT copies, half qn copies
  GPSIMD:          outT DMA (t-3)

Per-core DMA ~12.9MB (36us floor at 358GB/s); PE ~5us/block warm.
Host does cheap O(N*D) prep (sums/norms/layout packs) and the final
transpose + temp/||q|| scaling of the raw logits.
"""
import sys

sys.path.insert(0, "/opt/trn_rl_repo")

import numpy as np
import ml_dtypes

import concourse.bass as bass
import concourse.tile as tile
from concourse import bacc, mybir
from concourse.bass_utils import run_bass_kernel_spmd
from concourse.masks import make_identity

bf16 = mybir.dt.bfloat16
f32 = mybir.dt.float32

E, W, S, Q, D = 256, 20, 5, 300, 512
ALPHA = 37.0
NCORES = 8
EL = E // NCORES      # 32 episodes per core
BLK = 4               # episodes per block (packed at 32-partition offsets)
NBLK = EL // BLK      # 8 blocks
DC = D // 128         # 4 K-chunks over D
QCH = [(0, 128), (128, 128), (256, Q - 256)]  # q chunks (offset, count)

_BUILT = None


def _pin_act_table_set():
    """Make Bacc's ACT-table-load pass pick one covering set for Ln/Exp/Square.

    The pass walks activations and loads the first set containing the needed
    function; Ln's first set lacks Exp and vice versa, so alternating
    Ln/Exp/Square thrashes ACT_TABLE_LOAD (~1.3us each). Hide those functions
    from every set except natural_log_exp_and_others (set *indices* are
    preserved — contents of the real act_info.json are untouched).
    """
    import concourse.bacc as bacc_mod
    from concourse import hw_specs

    if getattr(bacc_mod, "_act_tables_pinned", False):
        return
    orig = hw_specs.get_activation_tables
    pin = {
        mybir.ActivationFunctionType.Ln,
        mybir.ActivationFunctionType.Exp,
        mybir.ActivationFunctionType.Square,
    }
    keep = "natural_log_exp_and_others"

    def pinned(arch):
        tabs = orig(arch)
        return {
            name: set(fns) if name == keep else (set(fns) - pin)
            for name, fns in tabs.items()
        }

    bacc_mod.get_activation_tables = pinned
    bacc_mod._act_tables_pinned = True


def _build():
    _pin_act_table_set()
    nc = bacc.Bacc("TRN2", target_bir_lowering=False, debug=False)

    qT = nc.declare_dram_parameter("qT", [128, NBLK, DC * BLK * Q], bf16, isOutput=False)
    mTs = nc.declare_dram_parameter("mTs", [128, NBLK, DC * BLK * W], bf16, isOutput=False)
    ssum_p = nc.declare_dram_parameter("ssum_p", [BLK * W, NBLK, D], bf16, isOutput=False)
    nq2hl = nc.declare_dram_parameter("nq2hl", [2 * BLK, NBLK, Q], bf16, isOutput=False)
    nm2b = nc.declare_dram_parameter("nm2b", [128, NBLK], f32, isOutput=False)
    sel8 = nc.declare_dram_parameter("sel8", [2 * BLK, 128], bf16, isOutput=False)
    sel80 = nc.declare_dram_parameter("sel80", [BLK * W, 128], bf16, isOutput=False)
    outT = nc.declare_dram_parameter("outT", [128, NBLK, Q], bf16, isOutput=True)

    with tile.TileContext(nc) as tc:
        with tc.tile_pool(name="const", bufs=1) as const, \
             tc.tile_pool(name="inp", bufs=2) as inp, \
             tc.tile_pool(name="qn", bufs=2) as qnp, \
             tc.tile_pool(name="mid", bufs=2) as mid, \
             tc.tile_pool(name="psG", bufs=2, space="PSUM") as psG, \
             tc.tile_pool(name="psPR", bufs=2, space="PSUM") as psPR, \
             tc.tile_pool(name="psLG", bufs=1, space="PSUM") as psLG, \
             tc.tile_pool(name="psQT", bufs=2, space="PSUM") as psQT, \
             tc.tile_pool(name="psT", bufs=1, space="PSUM") as psT:
            # PSUM banks: g x2 + pr x2 + lgT x1 + qtpA x2 + tp x1 = 8

            # ---- constants (loaded once) ----
            nm2b_t = const.tile([128, NBLK], f32)
            nc.sync.dma_start(out=nm2b_t, in_=nm2b[:, :])
            sel8_t = const.tile([2 * BLK, 128], bf16)
            nc.sync.dma_start(out=sel8_t, in_=sel8[:, :])
            sel80_t = const.tile([BLK * W, 128], bf16)
            nc.sync.dma_start(out=sel80_t, in_=sel80[:, :])
            ident = const.tile([128, 128], bf16)
            make_identity(nc, ident)
            # PE warmup: dense dummy matmuls during the DMA ramp so the HAM
            # un-throttles (1.2 -> 2.4 GHz) before the first real block.
            wups = psLG.tile([128, 128], f32, tag="lgT")
            for _ in range(80):
                nc.tensor.matmul(wups, ident, ident, start=True, stop=True)
            wupd = const.tile([128, 128], bf16)
            nc.vector.tensor_copy(wupd, wups)

            def s_dma(b, st):
                """input DMAs for block b."""
                qTt = inp.tile([128, DC, BLK, Q], bf16, tag="qTt", bufs=4)
                nc.sync.dma_start(
                    out=qTt,
                    in_=qT[:, b, :].rearrange("p (c j q) -> p c j q", c=DC, j=BLK),
                )
                mTs_t = inp.tile([128, DC, BLK, W], bf16, tag="mTs", bufs=3)
                nc.sync.dma_start(
                    out=mTs_t,
                    in_=mTs[:, b, :].rearrange("p (c j w) -> p c j w", c=DC, j=BLK),
                )
                ssum_t = inp.tile([BLK * W, D], bf16, tag="ssum", bufs=3)
                nc.sync.dma_start(out=ssum_t, in_=ssum_p[:, b, :])
                nq2_t = inp.tile([2 * BLK, Q], bf16, tag="nq2", bufs=3)
                nc.sync.dma_start(out=nq2_t, in_=nq2hl[:, b, :])
                st.update(qTt=qTt, mTs_t=mTs_t, ssum_t=ssum_t, nq2_t=nq2_t)

            def s_g(b, st):
                """G psum = nq2 (K=8 selector) - 2*mean.T @ q, col-tiled."""
                qTt, mTs_t, nq2_t = st["qTt"], st["mTs_t"], st["nq2_t"]
                g = psG.tile([128, Q], f32, tag="g")
                # emit in waves (all 4 col-groups adjacent) so the PE streams
                # the four 32-col chains concurrently
                for j in range(BLK):
                    nc.tensor.matmul(
                        g[32 * j:32 * j + 32, :],
                        sel8_t[:, 32 * j:32 * j + 32], nq2_t,
                        start=True, stop=False,
                        tile_position=(0, 32 * j),
                    )
                for c in range(DC):
                    for j in range(BLK):
                        nc.tensor.matmul(
                            g[32 * j:32 * j + W, :],
                            mTs_t[:, c, j, :], qTt[:, c, j, :],
                            start=False, stop=(c == DC - 1),
                            tile_position=(0, 32 * j),
                        )
                st["g"] = g

            def s_softmax_act(b, st):
                """dist chain from G psum (ACT half)."""
                g = st["g"]
                lnv = mid.tile([128, Q], f32, tag="lnv")
                nc.scalar.activation(out=lnv, in_=g,
                                     func=mybir.ActivationFunctionType.Ln,
                                     bias=nm2b_t[:, b:b + 1], scale=1.0)
                dist = mid.tile([128, Q], f32, tag="dist")
                nc.scalar.activation(out=dist, in_=lnv,
                                     func=mybir.ActivationFunctionType.Exp,
                                     bias=0.0, scale=0.5)
                wexp = mid.tile([128, Q], f32, tag="wexp")
                sums = mid.tile([128, 1], f32, tag="sums")
                nc.scalar.activation(out=wexp, in_=dist,
                                     func=mybir.ActivationFunctionType.Exp,
                                     bias=0.0, scale=-1.0, accum_out=sums)
                st.update(wexp=wexp, sums=sums)

            def s_softmax_dve(b, st):
                """softmax normalization (DVE half): w37 = wexp * (37/sums)."""
                recip = mid.tile([128, 1], f32, tag="recip")
                nc.vector.reciprocal(recip, st["sums"])
                w37 = mid.tile([128, Q], bf16, tag="w37")
                nc.vector.tensor_scalar(
                    out=w37, in0=st["wexp"], scalar1=recip, scalar2=ALPHA,
                    op0=mybir.AluOpType.mult, op1=mybir.AluOpType.mult,
                )
                st["w37"] = w37

            def s_qn_alloc(b, st):
                qn_t = qnp.tile([128, len(QCH), BLK, D], bf16, tag="qn")
                st["qn_t"] = qn_t

            def s_qtrans_j(b, st, j):
                """on-chip transpose of episode j's qT -> qn[:, :, j, :]."""
                qTt, qn_t = st["qTt"], st["qn_t"]
                # chunks 0,1 (cnt=128): 8 transposes -> one merged 2KB-psum copy
                tpA = psQT.tile([128, 2, DC, 128], bf16, tag="qtpA")
                for a in range(2):
                    q0, cnt = QCH[a]
                    for c in range(DC):
                        nc.tensor.transpose(
                            tpA[:, a, c, :], qTt[:, c, j, q0:q0 + cnt], ident)
                srcA = tpA.rearrange("p a c d -> p a (c d)")
                dstA = qn_t[:, 0:2, j, :]
                if j != 2:
                    nc.vector.tensor_copy(dstA, srcA)
                else:
                    nc.scalar.copy(dstA, srcA)
                # chunk 2 (cnt=44): shares the 'tp' psum bank
                q0, cnt = QCH[2]
                tpB = psT.tile([128, DC, 128], bf16, tag="tp")
                for c in range(DC):
                    nc.tensor.transpose(
                        tpB[0:cnt, c, :], qTt[:, c, j, q0:q0 + cnt], ident)
                srcB = tpB[0:cnt, :, :].rearrange("p c d -> p (c d)")
                dstB = qn_t[0:cnt, 2, j, :]
                if j < 2:
                    nc.scalar.copy(dstB, srcB)
                else:
                    nc.vector.tensor_copy(dstB, srcB)

            def s_wt(b, st):
                """wT transposes + merged copies."""
                w37 = st["w37"]
                wTps = psT.tile([128, len(QCH), 128], bf16, tag="tp")
                for ci, (q0, cnt) in enumerate(QCH):
                    nc.tensor.transpose(wTps[0:cnt, ci, :], w37[:, q0:q0 + cnt], ident)
                wTsb = mid.tile([128, len(QCH), 128], bf16, tag="wTsb")
                nc.vector.tensor_copy(wTsb[:, 0:2, :], wTps[:, 0:2, :])
                nc.vector.tensor_copy(wTsb[0:QCH[2][1], 2, :], wTps[0:QCH[2][1], 2, :])
                st["wTsb"] = wTsb

            def s_m2(b, st):
                """proto psum: ssum (K=80 selector) + 37*pooled; n2 via DVE."""
                wTsb, qn_t, ssum_t = st["wTsb"], st["qn_t"], st["ssum_t"]
                pr = psPR.tile([128, D], f32, tag="pr")
                for j in range(BLK):
                    nc.tensor.matmul(
                        pr[32 * j:32 * j + 32, :],
                        sel80_t[:, 32 * j:32 * j + 32], ssum_t,
                        start=True, stop=False,
                        tile_position=(0, 32 * j),
                    )
                for ci, (q0, cnt) in enumerate(QCH):
                    for j in range(BLK):
                        nc.tensor.matmul(
                            pr[32 * j:32 * j + W, :],
                            wTsb[0:cnt, ci, 32 * j:32 * j + W],
                            qn_t[0:cnt, ci, j, :],
                            start=False, stop=(ci == len(QCH) - 1),
                            tile_position=(0, 32 * j),
                        )
                # unnormalized proton: the 1/||proto|| lands on the logits rows
                proton = mid.tile([128, D], bf16, tag="proton")
                nc.vector.tensor_copy(proton, pr)
                sqdump = mid.tile([128, D], bf16, tag="sqdump")
                n2 = mid.tile([128, 1], f32, tag="n2")
                nc.scalar.activation(out=sqdump, in_=proton,
                                     func=mybir.ActivationFunctionType.Square,
                                     bias=0.0, scale=1.0, accum_out=n2)
                lnn = mid.tile([128, 1], f32, tag="lnn")
                nc.scalar.activation(out=lnn, in_=n2,
                                     func=mybir.ActivationFunctionType.Ln,
                                     bias=0.0, scale=1.0)
                rstd = mid.tile([128, 1], f32, tag="rstd")
                nc.scalar.activation(out=rstd, in_=lnn,
                                     func=mybir.ActivationFunctionType.Exp,
                                     bias=0.0, scale=-0.5)
                st.update(proton=proton, rstd=rstd)

            def s_pt(b, st):
                """protonT transposes + one merged copy."""
                ptps = psT.tile([128, DC, 128], bf16, tag="tp")
                for c in range(DC):
                    nc.tensor.transpose(ptps[:, c, :], st["proton"][:, 128 * c:128 * (c + 1)], ident)
                ptsb = mid.tile([128, DC, 128], bf16, tag="ptsb")
                nc.vector.tensor_copy(ptsb, ptps)
                st["ptsb"] = ptsb

            def s_m3(b, st):
                """logits matmul (transposed layout)."""
                ptsb, qTt = st["ptsb"], st["qTt"]
                lgT = psLG.tile([128, Q], f32, tag="lgT")
                for c in range(DC):
                    for j in range(BLK):
                        nc.tensor.matmul(
                            lgT[32 * j:32 * j + W, :],
                            ptsb[:, c, 32 * j:32 * j + W],
                            qTt[:, c, j, :],
                            start=(c == 0), stop=(c == DC - 1),
                            tile_position=(0, 32 * j),
                        )
                st["lgT"] = lgT

            def s_out(b, st):
                """scale rows by 1/||proto|| while copying psum->sbuf; DMA out."""
                lgsb = mid.tile([128, Q], bf16, tag="lgsb")
                nc.vector.tensor_scalar_mul(out=lgsb, in0=st["lgT"], scalar1=st["rstd"])
                nc.gpsimd.dma_start(out=outT[:, b, :], in_=lgsb)

            # 4-deep software pipeline. PE alternates two matmul-mode clumps
            # and two transpose-mode clumps per iteration: mode switches
            # flush the PE pipeline (~0.3us each) so transposes are batched,
            # but each transpose clump stays under the ~3.4us HAM window so
            # the clock gate keeps seeing regular-matmul activity.
            # PE/iter t: [G_t M2_{t-1}] [qj0 qj1] [M3_{t-3}] [qj2 qj3 pT_{t-2} wT_t]
            sts = {}
            for t in range(NBLK + 3):
                if t < NBLK:
                    sts[t] = {}
                    s_dma(t, sts[t])
                    s_g(t, sts[t])
                if 0 <= t - 1 < NBLK:
                    s_m2(t - 1, sts[t - 1])
                if t < NBLK:
                    s_softmax_act(t, sts[t])
                    s_qn_alloc(t, sts[t])
                    s_qtrans_j(t, sts[t], 0)
                    s_qtrans_j(t, sts[t], 1)
                    s_softmax_dve(t, sts[t])
                if 0 <= t - 3 < NBLK:
                    s_m3(t - 3, sts[t - 3])
                    s_out(t - 3, sts[t - 3])
                if t < NBLK:
                    s_qtrans_j(t, sts[t], 2)
                    s_qtrans_j(t, sts[t], 3)
                if 0 <= t - 2 < NBLK:
                    s_pt(t - 2, sts[t - 2])
                if t < NBLK:
                    s_wt(t, sts[t])
                if 0 <= t - 4 < NBLK:
                    del sts[t - 4]

    nc.finalize()
    return nc


def _get_built():
    global _BUILT
    if _BUILT is None:
        _BUILT = _build()
    return _BUILT


def _prep_core_inputs(x_shot, x_query, temp):
    """x_shot [EL,W,S,D] f32, x_query [EL,Q,D] f32 -> input map for one core."""
    qTr = x_query.transpose(2, 0, 1).reshape(DC, 128, NBLK, BLK, Q)
    qTr = np.ascontiguousarray(qTr.transpose(1, 2, 0, 3, 4)).reshape(128, NBLK, DC * BLK * Q)
    qTr = qTr.astype(ml_dtypes.bfloat16)

    shot_sum = x_shot.sum(axis=2)                    # [EL, W, D] f32
    mean = shot_sum / S
    # ssum packed rows: k = 20*j + w  ->  episode BLK*b+j
    ssp = shot_sum.reshape(NBLK, BLK * W, D).transpose(1, 0, 2)
    ssp = np.ascontiguousarray(ssp).astype(ml_dtypes.bfloat16)
    # mTs[p, b, (c j w)] = -2 * mean[4b+j, w, 128c+p]
    m = (-2.0 * mean).reshape(NBLK, BLK, W, DC, 128)
    m = m.transpose(4, 0, 3, 1, 2).reshape(128, NBLK, DC * BLK * W)
    mTs = np.ascontiguousarray(m).astype(ml_dtypes.bfloat16)

    nq2 = np.einsum("eqd,eqd->eq", x_query.astype(np.float64),
                    x_query.astype(np.float64)).astype(np.float32)   # [EL, Q]
    nq2b = nq2.reshape(NBLK, BLK, Q)                 # [b, j, q]
    hi = nq2b.astype(ml_dtypes.bfloat16)
    lo = (nq2b - hi.astype(np.float32)).astype(ml_dtypes.bfloat16)
    nq2hl = np.concatenate([hi, lo], axis=1)         # [b, 2*BLK, q]
    nq2hl = np.ascontiguousarray(nq2hl.transpose(1, 0, 2))  # [2*BLK, b, q]

    nm2 = np.einsum("ewd,ewd->ew", mean, mean)       # [EL, W] f32
    nm2b = np.zeros((128, NBLK), np.float32)
    for b in range(NBLK):
        for j in range(BLK):
            nm2b[32 * j:32 * j + W, b] = nm2[BLK * b + j]

    return {
        "qT": qTr, "mTs": mTs, "ssum_p": ssp, "nq2hl": nq2hl, "nm2b": nm2b,
    }


def _consts():
    sel8 = np.zeros((2 * BLK, 128), np.float32)
    for j in range(BLK):
        sel8[j, 32 * j:32 * j + W] = 1.0       # hi rows
        sel8[BLK + j, 32 * j:32 * j + W] = 1.0  # lo rows
    sel80 = np.zeros((BLK * W, 128), np.float32)
    for j in range(BLK):
        for w in range(W):
            sel80[W * j + w, 32 * j + w] = 1.0
    return {
        "sel8": sel8.astype(ml_dtypes.bfloat16),
        "sel80": sel80.astype(ml_dtypes.bfloat16),
    }


def _run(x_shot, x_query, temp, trace=False):
    nc = _get_built()
    consts = _consts()
    in_maps = []
    for i in range(NCORES):
        sl = slice(i * EL, (i + 1) * EL)
        m = _prep_core_inputs(x_shot[sl], x_query[sl], temp)
        m.update(consts)
        in_maps.append(m)
    res = run_bass_kernel_spmd(
        nc, in_maps, list(range(NCORES)), trace=trace,
        tmpdir="/tmp/bass_trace_out" if trace else None,
    )
    out = np.empty((E, Q, W), np.float32)
    for i in range(NCORES):
        sl = slice(i * EL, (i + 1) * EL)
        nq2 = np.einsum("eqd,eqd->eq", x_query[sl].astype(np.float64),
                        x_query[sl].astype(np.float64)).astype(np.float32)
        qscale = (np.float32(temp) / np.sqrt(nq2))[:, :, None]   # [EL, Q, 1]
        raw = res.results[i]["outT"].astype(np.float32).reshape(4, 32, NBLK, Q)
        lg = raw[:, 0:W].transpose(2, 0, 1, 3).reshape(EL, W, Q) # [e, w, q]
        out[sl] = lg.transpose(0, 2, 1) * qscale
    return out, res


def kernel(x_shot, x_query, temp):
    x_shot = np.asarray(x_shot, dtype=np.float32)
    x_query = np.asarray(x_query, dtype=np.float32)
    out, _ = _run(x_shot, x_query, np.float32(temp))
    return out


def kernel_timed(x_shot, x_query, temp):
    x_shot = np.asarray(x_shot, dtype=np.float32)
    x_query = np.asarray(x_query, dtype=np.float32)
    out, res = _run(x_shot, x_query, np.float32(temp), trace=True)
    return out, res


# revision 21
# speedup vs baseline: 1.4666x; 1.0386x over previous
"""MetaBaseline (retrieval_knn) Trainium2 kernel.

Problem: E=256 episodes; per episode:
  shot_sum[W,D], shot_mean = mean over S shots
  dist[W,Q]   = ||shot_mean_w - q_q||_2
  weights     = softmax(-dist, axis=Q)
  pooled[W,D] = weights @ x_query
  proto       = l2norm(shot_sum + 37*pooled)       (the /42 cancels in l2norm)
  logits[Q,W] = temp * l2norm(x_query) @ proto.T

Sharding: pure data parallel over E across 8 NeuronCores (32 episodes/core).
On-device layout: blocks of 4 episodes packed on the partition dim at
32-partition offsets (col-tiled matmuls), so softmax/activation work runs on
[128, Q] tiles serving 4 episodes at once.

v2 design (vs v1): x_query ships ONCE (transposed layout qT only, 9.8MB/core);
the natural-layout copy needed by the pooled matmul is produced on-chip with
PE transposes (48 [128,<=128] transposes/block) + ACT/DVE psum->sbuf copies.
nq2 (per-query norms) broadcast into the dist psum via a K=8 selector matmul
(bf16 hi+lo rows for f32-grade accuracy); shot_sum added into the proto psum
via a K=80 selector matmul of packed rows. Software pipeline uses a 4-deep
skew so the PE never waits on the serial ACT softmax/l2norm chains:

  iter t PE order: G_t | qnatT_t (48 tp) | M3_{t-3} | wT_{t-1} | pT_{t-2} | M2_{t-1}
  ACT during t:    softmax_t (Ln/Exp/Exp+accum), l2norm_{t-1}, lgsb_{t-3}, half qn copies
  DVE during t:    recip_t, w37_t, proton_{t-1}, wT/pT copies, half qn copies
  GPSIMD:          outT DMA (t-3)

Per-core DMA ~12.9MB (36us floor at 358GB/s); PE ~5us/block warm.
Host does cheap O(N*D) prep (sums/norms/layout packs) and the final
transpose + temp/||q|| scaling of the raw logits.
"""
import sys

sys.path.insert(0, "/opt/trn_rl_repo")

import numpy as np
import ml_dtypes

import concourse.bass as bass
import concourse.tile as tile
from concourse import bacc, mybir
from concourse.bass_utils import run_bass_kernel_spmd
from concourse.masks import make_identity

bf16 = mybir.dt.bfloat16
f32 = mybir.dt.float32

E, W, S, Q, D = 256, 20, 5, 300, 512
ALPHA = 37.0
NCORES = 8
EL = E // NCORES      # 32 episodes per core
BLK = 4               # episodes per block (packed at 32-partition offsets)
NBLK = EL // BLK      # 8 blocks
DC = D // 128         # 4 K-chunks over D
QCH = [(0, 128), (128, 128), (256, Q - 256)]  # q chunks (offset, count)

_BUILT = None


def _pin_act_table_set():
    """Make Bacc's ACT-table-load pass pick one covering set for Ln/Exp/Square.

    The pass walks activations and loads the first set containing the needed
    function; Ln's first set lacks Exp and vice versa, so alternating
    Ln/Exp/Square thrashes ACT_TABLE_LOAD (~1.3us each). Hide those functions
    from every set except natural_log_exp_and_others (set *indices* are
    preserved — contents of the real act_info.json are untouched).
    """
    import concourse.bacc as bacc_mod
    from concourse import hw_specs

    if getattr(bacc_mod, "_act_tables_pinned", False):
        return
    orig = hw_specs.get_activation_tables
    pin = {
        mybir.ActivationFunctionType.Ln,
        mybir.ActivationFunctionType.Exp,
        mybir.ActivationFunctionType.Square,
    }
    keep = "natural_log_exp_and_others"

    def pinned(arch):
        tabs = orig(arch)
        return {
            name: set(fns) if name == keep else (set(fns) - pin)
            for name, fns in tabs.items()
        }

    bacc_mod.get_activation_tables = pinned
    bacc_mod._act_tables_pinned = True


def _build():
    _pin_act_table_set()
    nc = bacc.Bacc("TRN2", target_bir_lowering=False, debug=False)

    qT = nc.declare_dram_parameter("qT", [128, NBLK, DC * BLK * Q], bf16, isOutput=False)
    mTs = nc.declare_dram_parameter("mTs", [128, NBLK, DC * BLK * W], bf16, isOutput=False)
    ssum_p = nc.declare_dram_parameter("ssum_p", [128, NBLK, D], bf16, isOutput=False)
    nq2hl = nc.declare_dram_parameter("nq2hl", [2 * BLK, NBLK, Q], bf16, isOutput=False)
    nm2b = nc.declare_dram_parameter("nm2b", [128, NBLK], f32, isOutput=False)
    sel8 = nc.declare_dram_parameter("sel8", [2 * BLK, 128], bf16, isOutput=False)
    outT = nc.declare_dram_parameter("outT", [128, NBLK, Q], bf16, isOutput=True)

    with tile.TileContext(nc) as tc:
        with tc.tile_pool(name="const", bufs=1) as const, \
             tc.tile_pool(name="inp", bufs=2) as inp, \
             tc.tile_pool(name="qn", bufs=2) as qnp, \
             tc.tile_pool(name="mid", bufs=2) as mid, \
             tc.tile_pool(name="psG", bufs=2, space="PSUM") as psG, \
             tc.tile_pool(name="psPR", bufs=2, space="PSUM") as psPR, \
             tc.tile_pool(name="psLG", bufs=2, space="PSUM") as psLG, \
             tc.tile_pool(name="psQT", bufs=2, space="PSUM") as psQT:
            # PSUM banks: g x2 + pr x2 + lgT x2 + qtpA x2 = 8

            # ---- constants (loaded once) ----
            nm2b_t = const.tile([128, NBLK], f32)
            nc.sync.dma_start(out=nm2b_t, in_=nm2b[:, :])
            sel8_t = const.tile([2 * BLK, 128], bf16)
            nc.sync.dma_start(out=sel8_t, in_=sel8[:, :])
            ident = const.tile([128, 128], bf16)
            make_identity(nc, ident)
            # PE warmup: dense dummy matmuls during the DMA ramp so the HAM
            # un-throttles (1.2 -> 2.4 GHz) before the first real block.
            wups = psLG.tile([128, 128], f32, tag="lgT")
            for _ in range(80):
                nc.tensor.matmul(wups, ident, ident, start=True, stop=True)
            wupd = const.tile([128, 128], bf16)
            nc.vector.tensor_copy(wupd, wups)

            def s_dma(b, st):
                """input DMAs for block b."""
                qTt = inp.tile([128, DC, BLK, Q], bf16, tag="qTt", bufs=4)
                nc.sync.dma_start(
                    out=qTt,
                    in_=qT[:, b, :].rearrange("p (c j q) -> p c j q", c=DC, j=BLK),
                )
                mTs_t = inp.tile([128, DC, BLK, W], bf16, tag="mTs", bufs=3)
                nc.sync.dma_start(
                    out=mTs_t,
                    in_=mTs[:, b, :].rearrange("p (c j w) -> p c j w", c=DC, j=BLK),
                )
                ssum_t = inp.tile([128, D], bf16, tag="ssum", bufs=3)
                nc.sync.dma_start(out=ssum_t, in_=ssum_p[:, b, :])
                nq2_t = inp.tile([2 * BLK, Q], bf16, tag="nq2", bufs=3)
                nc.sync.dma_start(out=nq2_t, in_=nq2hl[:, b, :])
                st.update(qTt=qTt, mTs_t=mTs_t, ssum_t=ssum_t, nq2_t=nq2_t)

            def s_g(b, st):
                """G psum = nq2 (K=8 selector) - 2*mean.T @ q, col-tiled."""
                qTt, mTs_t, nq2_t = st["qTt"], st["mTs_t"], st["nq2_t"]
                g = psG.tile([128, Q], f32, tag="g")
                # emit in waves (all 4 col-groups adjacent) so the PE streams
                # the four 32-col chains concurrently
                for j in range(BLK):
                    nc.tensor.matmul(
                        g[32 * j:32 * j + 32, :],
                        sel8_t[:, 32 * j:32 * j + 32], nq2_t,
                        start=True, stop=False,
                        tile_position=(0, 32 * j),
                    )
                for c in range(DC):
                    for j in range(BLK):
                        nc.tensor.matmul(
                            g[32 * j:32 * j + W, :],
                            mTs_t[:, c, j, :], qTt[:, c, j, :],
                            start=False, stop=(c == DC - 1),
                            tile_position=(0, 32 * j),
                        )
                st["g"] = g

            def s_softmax_act(b, st):
                """dist chain from G psum (ACT half)."""
                g = st["g"]
                lnv = mid.tile([128, Q], f32, tag="lnv")
                nc.scalar.activation(out=lnv, in_=g,
                                     func=mybir.ActivationFunctionType.Ln,
                                     bias=nm2b_t[:, b:b + 1], scale=1.0)
                dist = mid.tile([128, Q], f32, tag="dist")
                nc.scalar.activation(out=dist, in_=lnv,
                                     func=mybir.ActivationFunctionType.Exp,
                                     bias=0.0, scale=0.5)
                wexp = mid.tile([128, Q], f32, tag="wexp")
                sums = mid.tile([128, 1], f32, tag="sums")
                nc.scalar.activation(out=wexp, in_=dist,
                                     func=mybir.ActivationFunctionType.Exp,
                                     bias=0.0, scale=-1.0, accum_out=sums)
                st.update(wexp=wexp, sums=sums)

            def s_softmax_dve(b, st):
                """softmax normalization (DVE half): w37 = wexp * (37/sums)."""
                recip = mid.tile([128, 1], f32, tag="recip")
                nc.vector.reciprocal(recip, st["sums"])
                w37 = mid.tile([128, Q], bf16, tag="w37")
                nc.vector.tensor_scalar(
                    out=w37, in0=st["wexp"], scalar1=recip, scalar2=ALPHA,
                    op0=mybir.AluOpType.mult, op1=mybir.AluOpType.mult,
                )
                st["w37"] = w37

            def s_qn_alloc(b, st):
                qn_t = qnp.tile([128, len(QCH), BLK, D], bf16, tag="qn")
                st["qn_t"] = qn_t

            def s_qtrans_j(b, st, j):
                """on-chip transpose of episode j's qT -> qn[:, :, j, :]."""
                qTt, qn_t = st["qTt"], st["qn_t"]
                # chunks 0,1 (cnt=128): 8 transposes -> one merged 2KB-psum copy
                tpA = psQT.tile([128, 2, DC, 128], bf16, tag="qtpA")
                for a in range(2):
                    q0, cnt = QCH[a]
                    for c in range(DC):
                        nc.tensor.transpose(
                            tpA[:, a, c, :], qTt[:, c, j, q0:q0 + cnt], ident)
                srcA = tpA.rearrange("p a c d -> p a (c d)")
                dstA = qn_t[:, 0:2, j, :]
                if j != 2:
                    nc.vector.tensor_copy(dstA, srcA)
                else:
                    nc.scalar.copy(dstA, srcA)
                # chunk 2 (cnt=44): shares the 'tp' psum bank
                q0, cnt = QCH[2]
                tpB = psQT.tile([128, DC, 128], bf16, tag="qtpA")
                for c in range(DC):
                    nc.tensor.transpose(
                        tpB[0:cnt, c, :], qTt[:, c, j, q0:q0 + cnt], ident)
                srcB = tpB[0:cnt, :, :].rearrange("p c d -> p (c d)")
                dstB = qn_t[0:cnt, 2, j, :]
                if j < 2:
                    nc.scalar.copy(dstB, srcB)
                else:
                    nc.vector.tensor_copy(dstB, srcB)

            def s_wt(b, st):
                """wT transposes + merged copies."""
                w37 = st["w37"]
                wTps = psQT.tile([128, len(QCH), 128], bf16, tag="qtpA")
                for ci, (q0, cnt) in enumerate(QCH):
                    nc.tensor.transpose(wTps[0:cnt, ci, :], w37[:, q0:q0 + cnt], ident)
                wTsb = mid.tile([128, len(QCH), 128], bf16, tag="wTsb")
                nc.vector.tensor_copy(wTsb[:, 0:2, :], wTps[:, 0:2, :])
                nc.vector.tensor_copy(wTsb[0:QCH[2][1], 2, :], wTps[0:QCH[2][1], 2, :])
                st["wTsb"] = wTsb

            def s_m2(b, st):
                """proto psum: ssum (K=80 selector) + 37*pooled; n2 via DVE."""
                wTsb, qn_t, ssum_t = st["wTsb"], st["qn_t"], st["ssum_t"]
                pr = psPR.tile([128, D], f32, tag="pr")
                for ci, (q0, cnt) in enumerate(QCH):
                    for j in range(BLK):
                        nc.tensor.matmul(
                            pr[32 * j:32 * j + W, :],
                            wTsb[0:cnt, ci, 32 * j:32 * j + W],
                            qn_t[0:cnt, ci, j, :],
                            start=(ci == 0), stop=(ci == len(QCH) - 1),
                            tile_position=(0, 32 * j),
                        )
                # proton = 37*pooled + shot_sum, unnormalized (psum + sbuf add);
                # the 1/||proto|| lands on the logits rows at s_out
                proton = mid.tile([128, D], bf16, tag="proton")
                nc.vector.tensor_tensor(out=proton, in0=pr, in1=ssum_t,
                                        op=mybir.AluOpType.add)
                sqdump = mid.tile([128, D], bf16, tag="sqdump")
                n2 = mid.tile([128, 1], f32, tag="n2")
                nc.scalar.activation(out=sqdump, in_=proton,
                                     func=mybir.ActivationFunctionType.Square,
                                     bias=0.0, scale=1.0, accum_out=n2)
                lnn = mid.tile([128, 1], f32, tag="lnn")
                nc.scalar.activation(out=lnn, in_=n2,
                                     func=mybir.ActivationFunctionType.Ln,
                                     bias=0.0, scale=1.0)
                rstd = mid.tile([128, 1], f32, tag="rstd")
                nc.scalar.activation(out=rstd, in_=lnn,
                                     func=mybir.ActivationFunctionType.Exp,
                                     bias=0.0, scale=-0.5)
                st.update(proton=proton, rstd=rstd)

            def s_pt(b, st):
                """protonT transposes + one merged copy."""
                ptps = psQT.tile([128, DC, 128], bf16, tag="qtpA")
                for c in range(DC):
                    nc.tensor.transpose(ptps[:, c, :], st["proton"][:, 128 * c:128 * (c + 1)], ident)
                ptsb = mid.tile([128, DC, 128], bf16, tag="ptsb")
                nc.vector.tensor_copy(ptsb, ptps)
                st["ptsb"] = ptsb

            def s_m3(b, st):
                """logits matmul (transposed layout)."""
                ptsb, qTt = st["ptsb"], st["qTt"]
                lgT = psLG.tile([128, Q], f32, tag="lgT")
                for c in range(DC):
                    for j in range(BLK):
                        nc.tensor.matmul(
                            lgT[32 * j:32 * j + W, :],
                            ptsb[:, c, 32 * j:32 * j + W],
                            qTt[:, c, j, :],
                            start=(c == 0), stop=(c == DC - 1),
                            tile_position=(0, 32 * j),
                        )
                st["lgT"] = lgT

            def s_out(b, st):
                """scale rows by 1/||proto|| while copying psum->sbuf; DMA out."""
                lgsb = mid.tile([128, Q], bf16, tag="lgsb")
                nc.vector.tensor_scalar_mul(out=lgsb, in0=st["lgT"], scalar1=st["rstd"])
                nc.gpsimd.dma_start(out=outT[:, b, :], in_=lgsb)

            # 4-deep software pipeline. PE alternates two matmul-mode clumps
            # and two transpose-mode clumps per iteration: mode switches
            # flush the PE pipeline (~0.3us each) so transposes are batched,
            # but each transpose clump stays under the ~3.4us HAM window so
            # the clock gate keeps seeing regular-matmul activity.
            # PE/iter t: [G_t M2_{t-1}] [qj0 qj1] [M3_{t-3}] [qj2 qj3 pT_{t-2} wT_t]
            sts = {}
            for t in range(NBLK + 3):
                if t < NBLK:
                    sts[t] = {}
                    s_dma(t, sts[t])
                    s_g(t, sts[t])
                if 0 <= t - 1 < NBLK:
                    s_m2(t - 1, sts[t - 1])
                if t < NBLK:
                    s_softmax_act(t, sts[t])
                    s_qn_alloc(t, sts[t])
                    s_qtrans_j(t, sts[t], 0)
                    s_qtrans_j(t, sts[t], 1)
                    s_softmax_dve(t, sts[t])
                if 0 <= t - 3 < NBLK:
                    s_m3(t - 3, sts[t - 3])
                    s_out(t - 3, sts[t - 3])
                if t < NBLK:
                    s_wt(t, sts[t])
                    s_qtrans_j(t, sts[t], 2)
                    s_qtrans_j(t, sts[t], 3)
                if 0 <= t - 2 < NBLK:
                    s_pt(t - 2, sts[t - 2])
                if 0 <= t - 4 < NBLK:
                    del sts[t - 4]

    nc.finalize()
    return nc


def _get_built():
    global _BUILT
    if _BUILT is None:
        _BUILT = _build()
    return _BUILT


def _prep_core_inputs(x_shot, x_query, temp):
    """x_shot [EL,W,S,D] f32, x_query [EL,Q,D] f32 -> input map for one core."""
    qTr = x_query.transpose(2, 0, 1).reshape(DC, 128, NBLK, BLK, Q)
    qTr = np.ascontiguousarray(qTr.transpose(1, 2, 0, 3, 4)).reshape(128, NBLK, DC * BLK * Q)
    qTr = qTr.astype(ml_dtypes.bfloat16)

    shot_sum = x_shot.sum(axis=2)                    # [EL, W, D] f32
    mean = shot_sum / S
    ssp = np.zeros((128, NBLK, D), np.float32)
    for b in range(NBLK):
        for j in range(BLK):
            ssp[32 * j:32 * j + W, b, :] = shot_sum[BLK * b + j]
    ssp = ssp.astype(ml_dtypes.bfloat16)
    # mTs[p, b, (c j w)] = -2 * mean[4b+j, w, 128c+p]
    m = (-2.0 * mean).reshape(NBLK, BLK, W, DC, 128)
    m = m.transpose(4, 0, 3, 1, 2).reshape(128, NBLK, DC * BLK * W)
    mTs = np.ascontiguousarray(m).astype(ml_dtypes.bfloat16)

    nq2 = np.einsum("eqd,eqd->eq", x_query.astype(np.float64),
                    x_query.astype(np.float64)).astype(np.float32)   # [EL, Q]
    nq2b = nq2.reshape(NBLK, BLK, Q)                 # [b, j, q]
    hi = nq2b.astype(ml_dtypes.bfloat16)
    lo = (nq2b - hi.astype(np.float32)).astype(ml_dtypes.bfloat16)
    nq2hl = np.concatenate([hi, lo], axis=1)         # [b, 2*BLK, q]
    nq2hl = np.ascontiguousarray(nq2hl.transpose(1, 0, 2))  # [2*BLK, b, q]

    nm2 = np.einsum("ewd,ewd->ew", mean, mean)       # [EL, W] f32
    nm2b = np.zeros((128, NBLK), np.float32)
    for b in range(NBLK):
        for j in range(BLK):
            nm2b[32 * j:32 * j + W, b] = nm2[BLK * b + j]

    return {
        "qT": qTr, "mTs": mTs, "ssum_p": ssp, "nq2hl": nq2hl, "nm2b": nm2b,
    }


def _consts():
    sel8 = np.zeros((2 * BLK, 128), np.float32)
    for j in range(BLK):
        sel8[j, 32 * j:32 * j + W] = 1.0       # hi rows
        sel8[BLK + j, 32 * j:32 * j + W] = 1.0  # lo rows
    return {
        "sel8": sel8.astype(ml_dtypes.bfloat16),
    }


def _run(x_shot, x_query, temp, trace=False):
    nc = _get_built()
    consts = _consts()
    in_maps = []
    for i in range(NCORES):
        sl = slice(i * EL, (i + 1) * EL)
        m = _prep_core_inputs(x_shot[sl], x_query[sl], temp)
        m.update(consts)
        in_maps.append(m)
    res = run_bass_kernel_spmd(
        nc, in_maps, list(range(NCORES)), trace=trace,
        tmpdir="/tmp/bass_trace_out" if trace else None,
    )
    out = np.empty((E, Q, W), np.float32)
    for i in range(NCORES):
        sl = slice(i * EL, (i + 1) * EL)
        nq2 = np.einsum("eqd,eqd->eq", x_query[sl].astype(np.float64),
                        x_query[sl].astype(np.float64)).astype(np.float32)
        qscale = (np.float32(temp) / np.sqrt(nq2))[:, :, None]   # [EL, Q, 1]
        raw = res.results[i]["outT"].astype(np.float32).reshape(4, 32, NBLK, Q)
        lg = raw[:, 0:W].transpose(2, 0, 1, 3).reshape(EL, W, Q) # [e, w, q]
        out[sl] = lg.transpose(0, 2, 1) * qscale
    return out, res


def kernel(x_shot, x_query, temp):
    x_shot = np.asarray(x_shot, dtype=np.float32)
    x_query = np.asarray(x_query, dtype=np.float32)
    out, _ = _run(x_shot, x_query, np.float32(temp))
    return out


def kernel_timed(x_shot, x_query, temp):
    x_shot = np.asarray(x_shot, dtype=np.float32)
    x_query = np.asarray(x_query, dtype=np.float32)
    out, res = _run(x_shot, x_query, np.float32(temp), trace=True)
    return out, res
